# revision 4
# baseline (speedup 1.0000x reference)
"""CrossAttentionFusion Trainium2 kernel.

Problem (per batch element b of 4, C=128 channels, N=4096 tokens):
    Q1 = wq1@hsi+bq1; K1 = wk1@msi+bk1; V1 = wv1@msi+bv1   (1x1 convs)
    Q2 = wq2@msi+bq2; K2 = wk2@hsi+bk2; V2 = wv2@hsi+bv2
    out1 = attn(Q1,K1,V1); out2 = attn(Q2,K2,V2)           (softmax over keys)
    g = sigmoid(wg@[hsi;msi]+bg)
    out = wp@(g*out1 + (1-g)*out2) + bp

Sharding: 8 cores = (b, query-half). Each core computes 2048 query columns
for one batch element; keys/values span all 4096 tokens. Host permutes the
token axis per core so its queries are the first 2048 columns (key order is
irrelevant to attention sums), so the SPMD program is offset-free.

Core dataflow (transposed attention, keys on partitions):
    sT[m,n] = K[:,m]^T Q[:,n]        via matmul(lhsT=K tile, rhs=Q)
    pT = exp(scale*sT)               ACT, direct from PSUM
    den[n] = sum_m pT[m,n]           via matmul(lhsT=ones[128,128]) -> bcast rows
    outU[c,n] = sum_m VT[m,c] pT[m,n] accumulated over key tiles in PSUM
    out = outU * (1/den) + bv        (V-bias folds through softmax exactly)
Gate uses sigmoid(z) = 0.5*tanh(0.5 z)+0.5; the 0.5 factors fold into wp.
Matmuls run as float32r (full-rate fp32); the V-producing convs stay
float32 (exact) since their N=128 free dim gets no fp32r speedup anyway.
"""

import sys

if "/opt/trn_rl_repo" not in sys.path:
    sys.path.insert(0, "/opt/trn_rl_repo")

from contextlib import ExitStack

import numpy as np

import concourse.bacc as bacc
import concourse.bass as bass  # noqa: F401
import concourse.tile as tile
from concourse import mybir

F32 = mybir.dt.float32
F32R = mybir.dt.float32r
C = 128
N_TOK = 4096
NQ = 2048
FD = 512  # matmul moving-operand max for 4-byte dtypes
CH = 1024  # query-chunk width (PSUM accumulator width)
SCALE = 1.0 / float(np.sqrt(np.float32(C)))

WEIGHT_NAMES = ["wq1T", "wk1T", "wv1T", "wq2T", "wk2T", "wv2T", "wgaT", "wgbT", "wpTs"]
BIAS_NAMES = ["bq1", "bk1", "bq2", "bk2", "bv1", "bv2", "bgh", "bp"]


def _r(ap):
    return ap.bitcast(F32R)


def build_program(n_tok=N_TOK, nq=NQ, ch=CH, fd=FD):
    mt = n_tok // 128  # key tiles
    nch = nq // ch  # query chunks
    spc = ch // fd  # matmul slices per chunk
    vtg = ch // 128  # VT tiles per eviction group

    nc = bacc.Bacc("TRN2", target_bir_lowering=False, debug=False)
    din = {}
    for name in ["x_h", "x_m"]:
        din[name] = nc.dram_tensor(name, [C, n_tok], F32, kind="ExternalInput").ap()
    for name in WEIGHT_NAMES:
        din[name] = nc.dram_tensor(name, [C, C], F32, kind="ExternalInput").ap()
    for name in BIAS_NAMES:
        din[name] = nc.dram_tensor(name, [C, 1], F32, kind="ExternalInput").ap()
    din["ones"] = nc.dram_tensor("ones", [C, C], F32, kind="ExternalInput").ap()
    out_d = nc.dram_tensor("out", [C, nq], F32, kind="ExternalOutput").ap()

    with ExitStack() as ctx:
        tc = ctx.enter_context(tile.TileContext(nc))
        const = ctx.enter_context(tc.tile_pool(name="const", bufs=1))
        big = ctx.enter_context(tc.tile_pool(name="big", bufs=1))
        ppool = ctx.enter_context(tc.tile_pool(name="ppool", bufs=3))
        rpool = ctx.enter_context(tc.tile_pool(name="rpool", bufs=2))
        stpool = ctx.enter_context(tc.tile_pool(name="stpool", bufs=2))
        ps_pool = ctx.enter_context(tc.tile_pool(name="ps", bufs=2, space="PSUM"))
        pacc_pool = ctx.enter_context(tc.tile_pool(name="pacc", bufs=2, space="PSUM"))

        # constants in
        w_sb = {}
        for name in WEIGHT_NAMES:
            w_sb[name] = const.tile([C, C], F32R, name=name)
            nc.sync.dma_start(out=w_sb[name][:], in_=_r(din[name][:]))
        b_sb = {}
        for name in BIAS_NAMES:
            b_sb[name] = const.tile([C, 1], F32, name=name)
            nc.sync.dma_start(out=b_sb[name][:], in_=din[name][:])
        ones_sb = const.tile([C, C], F32R, name="ones")
        nc.sync.dma_start(out=ones_sb[:], in_=_r(din["ones"][:]))

        # activations in (chunked so downstream convs can start early)
        xh_sb = big.tile([C, n_tok], F32R, name="xh")
        xm_sb = big.tile([C, n_tok], F32R, name="xm")
        for j in range(n_tok // fd):
            sl = slice(j * fd, (j + 1) * fd)
            nc.sync.dma_start(out=xh_sb[:, sl], in_=_r(din["x_h"][:, sl]))
            nc.sync.dma_start(out=xm_sb[:, sl], in_=_r(din["x_m"][:, sl]))

        K1_sb = big.tile([C, n_tok], F32R, name="K1")
        K2_sb = big.tile([C, n_tok], F32R, name="K2")
        VT1_sb = big.tile([C, n_tok], F32R, name="VT1")
        VT2_sb = big.tile([C, n_tok], F32R, name="VT2")
        Q1_sb = big.tile([C, nq], F32R, name="Q1")
        Q2_sb = big.tile([C, nq], F32R, name="Q2")
        o1_sb = big.tile([C, nq], F32R, name="o1")
        o2_sb = big.tile([C, nq], F32R, name="o2")
        t_sb = big.tile([C, nq], F32R, name="t")
        d_sb = big.tile([C, nq], F32R, name="d")

        def conv(dst_sb, wT_sb, x_sb, cols, bias_sb):
            # dst[:, :cols] = wT.T @ x[:, :cols] (+ bias per channel)
            for j in range(cols // ch):
                ps = ps_pool.tile([C, ch], F32, tag="ps")
                for s in range(spc):
                    sl = slice(j * ch + s * fd, j * ch + (s + 1) * fd)
                    nc.tensor.matmul(
                        ps[:, s * fd : (s + 1) * fd],
                        wT_sb[:],
                        x_sb[:, sl],
                        start=True,
                        stop=True,
                    )
                dsl = slice(j * ch, (j + 1) * ch)
                if bias_sb is None:
                    nc.vector.tensor_copy(dst_sb[:, dsl], ps[:])
                else:
                    nc.vector.tensor_scalar_add(dst_sb[:, dsl], ps[:], bias_sb[:])

        def vt_conv(dst_sb, x_sb, wvT_sb):
            # dst tile j holds V^T rows for tokens [128j, 128j+128): [tok, chan]
            for g in range(mt // vtg):
                ps = ps_pool.tile([C, ch], F32, tag="ps")
                for u in range(vtg):
                    j = g * vtg + u
                    nc.tensor.matmul(
                        ps[:, u * 128 : (u + 1) * 128],
                        x_sb[:, j * 128 : (j + 1) * 128],
                        wvT_sb[:],
                        start=True,
                        stop=True,
                    )
                nc.vector.tensor_copy(dst_sb[:, g * ch : (g + 1) * ch], ps[:])

        conv(K1_sb, w_sb["wk1T"], xm_sb, n_tok, b_sb["bk1"])
        conv(Q1_sb, w_sb["wq1T"], xh_sb, nq, b_sb["bq1"])
        vt_conv(VT1_sb, xm_sb, w_sb["wv1T"])
        conv(K2_sb, w_sb["wk2T"], xh_sb, n_tok, b_sb["bk2"])
        conv(Q2_sb, w_sb["wq2T"], xm_sb, nq, b_sb["bq2"])
        vt_conv(VT2_sb, xh_sb, w_sb["wv2T"])

        # gate: t = tanh(0.5*(wgA@xq_h + wgB@xq_m) + 0.5*bg)
        for j in range(nq // ch):
            ps = ps_pool.tile([C, ch], F32, tag="ps")
            for s in range(spc):
                sl = slice(j * ch + s * fd, j * ch + (s + 1) * fd)
                psl = ps[:, s * fd : (s + 1) * fd]
                nc.tensor.matmul(
                    psl, w_sb["wgaT"][:], xh_sb[:, sl], start=True, stop=False
                )
                nc.tensor.matmul(
                    psl, w_sb["wgbT"][:], xm_sb[:, sl], start=False, stop=True
                )
            nc.scalar.activation(
                t_sb[:, j * ch : (j + 1) * ch],
                ps[:],
                mybir.ActivationFunctionType.Tanh,
                bias=b_sb["bgh"][:],
                scale=0.5,
            )

        def attention(o_sb, Q_sb, K_sb, VT_sb, bv_sb):
            for cidx in range(nch):
                p_out = pacc_pool.tile([C, ch], F32, tag="acc")
                p_den = pacc_pool.tile([C, ch], F32, tag="acc")
                for m in range(mt):
                    ksl = slice(m * 128, (m + 1) * 128)
                    ps = ps_pool.tile([C, ch], F32, tag="ps")
                    for s in range(spc):
                        qsl = slice(cidx * ch + s * fd, cidx * ch + (s + 1) * fd)
                        nc.tensor.matmul(
                            ps[:, s * fd : (s + 1) * fd],
                            K_sb[:, ksl],
                            Q_sb[:, qsl],
                            start=True,
                            stop=True,
                        )
                    pt = ppool.tile([C, ch], F32R, tag="pt")
                    nc.scalar.activation(
                        pt[:], ps[:], mybir.ActivationFunctionType.Exp, scale=SCALE
                    )
                    first, last = m == 0, m == mt - 1
                    for s in range(spc):
                        ssl = slice(s * fd, (s + 1) * fd)
                        nc.tensor.matmul(
                            p_den[:, ssl],
                            ones_sb[:],
                            pt[:, ssl],
                            start=first,
                            stop=last,
                        )
                        nc.tensor.matmul(
                            p_out[:, ssl],
                            VT_sb[:, ksl],
                            pt[:, ssl],
                            start=first,
                            stop=last,
                        )
                rec = rpool.tile([C, ch], F32, tag="rec")
                nc.vector.reciprocal(rec[:], p_den[:])
                osl = slice(cidx * ch, (cidx + 1) * ch)
                nc.vector.tensor_mul(o_sb[:, osl], p_out[:], rec[:])
                nc.vector.tensor_scalar_add(o_sb[:, osl], o_sb[:, osl], bv_sb[:])

        attention(o1_sb, Q1_sb, K1_sb, VT1_sb, b_sb["bv1"])
        attention(o2_sb, Q2_sb, K2_sb, VT2_sb, b_sb["bv2"])

        # fused' = (o1+o2) + t*(o1-o2), in chunks to pipeline with proj
        # out = (0.5*wp)^T.T @ fused' + bp
        for j in range(nq // ch):
            sl = slice(j * ch, (j + 1) * ch)
            nc.vector.tensor_sub(d_sb[:, sl], o1_sb[:, sl], o2_sb[:, sl])
            nc.vector.tensor_add(o1_sb[:, sl], o1_sb[:, sl], o2_sb[:, sl])
            nc.vector.tensor_mul(d_sb[:, sl], d_sb[:, sl], t_sb[:, sl])
            nc.vector.tensor_add(o1_sb[:, sl], o1_sb[:, sl], d_sb[:, sl])
        for j in range(nq // fd):
            sl = slice(j * fd, (j + 1) * fd)
            ps = ps_pool.tile([C, ch], F32, tag="ps")
            nc.tensor.matmul(
                ps[:, :fd], w_sb["wpTs"][:], o1_sb[:, sl], start=True, stop=True
            )
            st = stpool.tile([C, fd], F32, tag="st")
            nc.vector.tensor_scalar_add(st[:], ps[:, :fd], b_sb["bp"][:])
            nc.sync.dma_start(out=out_d[:, sl], in_=st[:])

    nc.compile()
    return nc


def make_in_maps(hsi, msi, weights, n_cores=8):
    """Host-side sharding: core i handles (b=i//2, half=i%2); the token axis is
    rotated so the core's queries are columns [0, NQ)."""
    B = hsi.shape[0]
    hsi = np.ascontiguousarray(hsi.reshape(B, C, N_TOK), dtype=np.float32)
    msi = np.ascontiguousarray(msi.reshape(B, C, N_TOK), dtype=np.float32)
    in_maps = []
    for core in range(n_cores):
        b, h = core // 2, core % 2
        if h == 0:
            x_h, x_m = hsi[b], msi[b]
        else:
            x_h = np.concatenate([hsi[b][:, NQ:], hsi[b][:, :NQ]], axis=1)
            x_m = np.concatenate([msi[b][:, NQ:], msi[b][:, :NQ]], axis=1)
        m = {"x_h": np.ascontiguousarray(x_h), "x_m": np.ascontiguousarray(x_m)}
        m.update(weights)
        in_maps.append(m)
    return in_maps


def make_weight_map(
    wq1, bq1, wk1, bk1, wv1, bv1, wq2, bq2, wk2, bk2, wv2, bv2, wg, bg, wp, bp
):
    f = np.float32
    col = lambda v: np.ascontiguousarray(np.asarray(v, f).reshape(C, 1))
    tr = lambda w: np.ascontiguousarray(np.asarray(w, f).T)
    return {
        "wq1T": tr(wq1), "wk1T": tr(wk1), "wv1T": tr(wv1),
        "wq2T": tr(wq2), "wk2T": tr(wk2), "wv2T": tr(wv2),
        "wgaT": tr(np.asarray(wg, f)[:, :C]),
        "wgbT": tr(np.asarray(wg, f)[:, C:]),
        "wpTs": tr(0.5 * np.asarray(wp, f)),
        "bq1": col(bq1), "bk1": col(bk1), "bq2": col(bq2), "bk2": col(bk2),
        "bv1": col(bv1), "bv2": col(bv2), "bgh": col(0.5 * np.asarray(bg, f)),
        "bp": col(bp),
        "ones": np.ones((C, C), np.float32),
    }


_NC_CACHE = {}


def _get_program():
    if "nc" not in _NC_CACHE:
        _NC_CACHE["nc"] = build_program()
    return _NC_CACHE["nc"]


def run_on_cores(in_maps, trace=False, **kwargs):
    from concourse.bass_utils import run_bass_kernel_spmd

    nc = _get_program()
    return run_bass_kernel_spmd(
        nc, in_maps, core_ids=list(range(len(in_maps))), trace=trace, **kwargs
    )


def kernel(
    hsi, msi, wq1, bq1, wk1, bk1, wv1, bv1, wq2, bq2, wk2, bk2, wv2, bv2,
    wg, bg, wp, bp,
):
    B, _, H, W = hsi.shape
    weights = make_weight_map(
        wq1, bq1, wk1, bk1, wv1, bv1, wq2, bq2, wk2, bk2, wv2, bv2, wg, bg, wp, bp
    )
    in_maps = make_in_maps(np.asarray(hsi), np.asarray(msi), weights)
    res = run_on_cores(in_maps)
    out = np.zeros((B, C, N_TOK), dtype=np.float32)
    for core in range(8):
        b, h = core // 2, core % 2
        out[b][:, h * NQ : (h + 1) * NQ] = res.results[core]["out"]
    return out.reshape(B, C, H, W)


# revision 5
# speedup vs baseline: 1.2358x; 1.2358x over previous
"""CrossAttentionFusion Trainium2 kernel.

Problem (per batch element b of 4, C=128 channels, N=4096 tokens):
    Q1 = wq1@hsi+bq1; K1 = wk1@msi+bk1; V1 = wv1@msi+bv1   (1x1 convs)
    Q2 = wq2@msi+bq2; K2 = wk2@hsi+bk2; V2 = wv2@hsi+bv2
    out1 = attn(Q1,K1,V1); out2 = attn(Q2,K2,V2)           (softmax over keys)
    g = sigmoid(wg@[hsi;msi]+bg)
    out = wp@(g*out1 + (1-g)*out2) + bp

Sharding: 8 cores = (b, query-half). Each core computes 2048 query columns
for one batch element; keys/values span all 4096 tokens. Host permutes the
token axis per core so its queries are the first 2048 columns (key order is
irrelevant to attention sums), so the SPMD program is offset-free.

Core dataflow (transposed attention, keys on partitions):
    sT[m,n] = K[:,m]^T Q[:,n]        via matmul(lhsT=K tile, rhs=Q)
    pT = exp(scale*sT)               ACT, direct from PSUM
    den[n] = sum_m pT[m,n]           via matmul(lhsT=ones[128,128]) -> bcast rows
    outU[c,n] = sum_m VT[m,c] pT[m,n] accumulated over key tiles in PSUM
    out = outU * (1/den) + bv        (V-bias folds through softmax exactly)
Gate uses sigmoid(z) = 0.5*tanh(0.5 z)+0.5; the 0.5 factors fold into wp.
Matmuls run as float32r (full-rate fp32); the V-producing convs stay
float32 (exact) since their N=128 free dim gets no fp32r speedup anyway.
"""

import sys

if "/opt/trn_rl_repo" not in sys.path:
    sys.path.insert(0, "/opt/trn_rl_repo")

from contextlib import ExitStack

import numpy as np

import concourse.bacc as bacc
import concourse.bass as bass  # noqa: F401
import concourse.tile as tile
from concourse import mybir

F32 = mybir.dt.float32
F32R = mybir.dt.float32r
C = 128
N_TOK = 4096
NQ = 2048
FD = 512  # matmul moving-operand max for 4-byte dtypes
CH = 1024  # query-chunk width (PSUM accumulator width)
SCALE = 1.0 / float(np.sqrt(np.float32(C)))

WEIGHT_NAMES = ["wq1T", "wk1T", "wv1T", "wq2T", "wk2T", "wv2T", "wgaT", "wgbT", "wpTs"]
BIAS_NAMES = ["bq1", "bk1", "bq2", "bk2", "bv1", "bv2", "bgh", "bp"]


def _r(ap):
    return ap.bitcast(F32R)


def build_program(n_tok=N_TOK, nq=NQ, ch=CH, fd=FD):
    mt = n_tok // 128  # key tiles
    nch = nq // ch  # query chunks
    spc = ch // fd  # matmul slices per chunk
    vtg = ch // 128  # VT tiles per eviction group

    nc = bacc.Bacc("TRN2", target_bir_lowering=False, debug=False)
    din = {}
    for name in ["x_h", "x_m"]:
        din[name] = nc.dram_tensor(name, [C, n_tok], F32, kind="ExternalInput").ap()
    for name in WEIGHT_NAMES:
        din[name] = nc.dram_tensor(name, [C, C], F32, kind="ExternalInput").ap()
    for name in BIAS_NAMES:
        din[name] = nc.dram_tensor(name, [C, 1], F32, kind="ExternalInput").ap()
    din["ones"] = nc.dram_tensor("ones", [C, C], F32, kind="ExternalInput").ap()
    out_d = nc.dram_tensor("out", [C, nq], F32, kind="ExternalOutput").ap()

    with ExitStack() as ctx:
        tc = ctx.enter_context(tile.TileContext(nc))
        const = ctx.enter_context(tc.tile_pool(name="const", bufs=1))
        big = ctx.enter_context(tc.tile_pool(name="big", bufs=1))
        ppool = ctx.enter_context(tc.tile_pool(name="ppool", bufs=4))
        rpool = ctx.enter_context(tc.tile_pool(name="rpool", bufs=2))
        stpool = ctx.enter_context(tc.tile_pool(name="stpool", bufs=2))
        ps_pool = ctx.enter_context(tc.tile_pool(name="ps", bufs=2, space="PSUM"))
        pacc_pool = ctx.enter_context(tc.tile_pool(name="pacc", bufs=2, space="PSUM"))

        # constants in
        w_sb = {}
        for name in WEIGHT_NAMES:
            w_sb[name] = const.tile([C, C], F32R, name=name)
            nc.sync.dma_start(out=w_sb[name][:], in_=_r(din[name][:]))
        b_sb = {}
        for name in BIAS_NAMES:
            b_sb[name] = const.tile([C, 1], F32, name=name)
            nc.sync.dma_start(out=b_sb[name][:], in_=din[name][:])
        ones_sb = const.tile([C, C], F32R, name="ones")
        nc.sync.dma_start(out=ones_sb[:], in_=_r(din["ones"][:]))

        # activations in (chunked so downstream convs can start early)
        xh_sb = big.tile([C, n_tok], F32R, name="xh")
        xm_sb = big.tile([C, n_tok], F32R, name="xm")
        for j in range(n_tok // fd):
            sl = slice(j * fd, (j + 1) * fd)
            nc.sync.dma_start(out=xh_sb[:, sl], in_=_r(din["x_h"][:, sl]))
            nc.sync.dma_start(out=xm_sb[:, sl], in_=_r(din["x_m"][:, sl]))

        K1_sb = big.tile([C, n_tok], F32R, name="K1")
        K2_sb = big.tile([C, n_tok], F32R, name="K2")
        VT1_sb = big.tile([C, n_tok], F32R, name="VT1")
        VT2_sb = big.tile([C, n_tok], F32R, name="VT2")
        Q1_sb = big.tile([C, nq], F32R, name="Q1")
        Q2_sb = big.tile([C, nq], F32R, name="Q2")
        o1_sb = big.tile([C, nq], F32R, name="o1")
        o2_sb = big.tile([C, nq], F32R, name="o2")
        t_sb = big.tile([C, nq], F32R, name="t")
        d_sb = big.tile([C, nq], F32R, name="d")

        def conv(dst_sb, wT_sb, x_sb, cols, bias_sb):
            # dst[:, :cols] = wT.T @ x[:, :cols] (+ bias per channel)
            for j in range(cols // ch):
                ps = ps_pool.tile([C, ch], F32, tag="ps")
                for s in range(spc):
                    sl = slice(j * ch + s * fd, j * ch + (s + 1) * fd)
                    nc.tensor.matmul(
                        ps[:, s * fd : (s + 1) * fd],
                        wT_sb[:],
                        x_sb[:, sl],
                        start=True,
                        stop=True,
                    )
                dsl = slice(j * ch, (j + 1) * ch)
                if bias_sb is None:
                    nc.vector.tensor_copy(dst_sb[:, dsl], ps[:])
                else:
                    nc.vector.tensor_scalar_add(dst_sb[:, dsl], ps[:], bias_sb[:])

        def vt_conv(dst_sb, x_sb, wvT_sb):
            # dst tile j holds V^T rows for tokens [128j, 128j+128): [tok, chan]
            for g in range(mt // vtg):
                ps = ps_pool.tile([C, ch], F32, tag="ps")
                for u in range(vtg):
                    j = g * vtg + u
                    nc.tensor.matmul(
                        ps[:, u * 128 : (u + 1) * 128],
                        x_sb[:, j * 128 : (j + 1) * 128],
                        wvT_sb[:],
                        start=True,
                        stop=True,
                    )
                nc.vector.tensor_copy(dst_sb[:, g * ch : (g + 1) * ch], ps[:])

        conv(K1_sb, w_sb["wk1T"], xm_sb, n_tok, b_sb["bk1"])
        conv(Q1_sb, w_sb["wq1T"], xh_sb, nq, b_sb["bq1"])
        vt_conv(VT1_sb, xm_sb, w_sb["wv1T"])
        conv(K2_sb, w_sb["wk2T"], xh_sb, n_tok, b_sb["bk2"])
        conv(Q2_sb, w_sb["wq2T"], xm_sb, nq, b_sb["bq2"])
        vt_conv(VT2_sb, xh_sb, w_sb["wv2T"])

        # gate: t = tanh(0.5*(wgA@xq_h + wgB@xq_m) + 0.5*bg)
        for j in range(nq // ch):
            ps = ps_pool.tile([C, ch], F32, tag="ps")
            for s in range(spc):
                sl = slice(j * ch + s * fd, j * ch + (s + 1) * fd)
                psl = ps[:, s * fd : (s + 1) * fd]
                nc.tensor.matmul(
                    psl, w_sb["wgaT"][:], xh_sb[:, sl], start=True, stop=False
                )
                nc.tensor.matmul(
                    psl, w_sb["wgbT"][:], xm_sb[:, sl], start=False, stop=True
                )
            nc.scalar.activation(
                t_sb[:, j * ch : (j + 1) * ch],
                ps[:],
                mybir.ActivationFunctionType.Tanh,
                bias=b_sb["bgh"][:],
                scale=0.5,
            )

        def attention(o_sb, Q_sb, K_sb, VT_sb, bv_sb, chunk_post=None):
            # Software-pipelined: scores for key-tile m+1 are emitted before
            # the den/PV matmuls of tile m, so the PE never sits waiting on
            # ACT's exp of tile m (exp overlaps the next 6 matmuls).
            for cidx in range(nch):
                p_out = pacc_pool.tile([C, ch], F32, tag="acc")
                p_den = pacc_pool.tile([C, ch], F32, tag="acc")
                pts = {}

                def scores(m):
                    ksl = slice(m * 128, (m + 1) * 128)
                    ps = ps_pool.tile([C, ch], F32, tag="ps")
                    for s in range(spc):
                        qsl = slice(cidx * ch + s * fd, cidx * ch + (s + 1) * fd)
                        nc.tensor.matmul(
                            ps[:, s * fd : (s + 1) * fd],
                            K_sb[:, ksl],
                            Q_sb[:, qsl],
                            start=True,
                            stop=True,
                        )
                    pt = ppool.tile([C, ch], F32R, tag="pt")
                    nc.scalar.activation(
                        pt[:], ps[:], mybir.ActivationFunctionType.Exp, scale=SCALE
                    )
                    pts[m] = pt

                def accum(m):
                    ksl = slice(m * 128, (m + 1) * 128)
                    pt = pts.pop(m)
                    first, last = m == 0, m == mt - 1
                    for s in range(spc):
                        ssl = slice(s * fd, (s + 1) * fd)
                        nc.tensor.matmul(
                            p_den[:, ssl], ones_sb[:], pt[:, ssl],
                            start=first, stop=last,
                        )
                        nc.tensor.matmul(
                            p_out[:, ssl], VT_sb[:, ksl], pt[:, ssl],
                            start=first, stop=last,
                        )

                scores(0)
                for m in range(1, mt):
                    scores(m)
                    accum(m - 1)
                accum(mt - 1)

                rec = rpool.tile([C, ch], F32, tag="rec")
                nc.vector.reciprocal_approx_fast(rec[:], p_den[:])
                osl = slice(cidx * ch, (cidx + 1) * ch)
                nc.vector.tensor_mul(o_sb[:, osl], p_out[:], rec[:])
                nc.vector.tensor_scalar_add(o_sb[:, osl], o_sb[:, osl], bv_sb[:])
                if chunk_post is not None:
                    chunk_post(cidx)

        attention(o1_sb, Q1_sb, K1_sb, VT1_sb, b_sb["bv1"])

        def fuse_and_project(cidx):
            # fused' = (o1+o2) + t*(o1-o2);  out = (0.5*wp)^T.T @ fused' + bp
            sl = slice(cidx * ch, (cidx + 1) * ch)
            nc.vector.tensor_sub(d_sb[:, sl], o1_sb[:, sl], o2_sb[:, sl])
            nc.vector.tensor_add(o1_sb[:, sl], o1_sb[:, sl], o2_sb[:, sl])
            nc.vector.tensor_mul(d_sb[:, sl], d_sb[:, sl], t_sb[:, sl])
            nc.vector.tensor_add(o1_sb[:, sl], o1_sb[:, sl], d_sb[:, sl])
            for s in range(spc):
                psl = slice(cidx * ch + s * fd, cidx * ch + (s + 1) * fd)
                ps = ps_pool.tile([C, ch], F32, tag="ps")
                nc.tensor.matmul(
                    ps[:, :fd], w_sb["wpTs"][:], o1_sb[:, psl], start=True, stop=True
                )
                st = stpool.tile([C, fd], F32, tag="st")
                nc.vector.tensor_scalar_add(st[:], ps[:, :fd], b_sb["bp"][:])
                nc.sync.dma_start(out=out_d[:, psl], in_=st[:])

        attention(o2_sb, Q2_sb, K2_sb, VT2_sb, b_sb["bv2"], chunk_post=fuse_and_project)

    nc.compile()
    return nc


def make_in_maps(hsi, msi, weights, n_cores=8):
    """Host-side sharding: core i handles (b=i//2, half=i%2); the token axis is
    rotated so the core's queries are columns [0, NQ)."""
    B = hsi.shape[0]
    hsi = np.ascontiguousarray(hsi.reshape(B, C, N_TOK), dtype=np.float32)
    msi = np.ascontiguousarray(msi.reshape(B, C, N_TOK), dtype=np.float32)
    in_maps = []
    for core in range(n_cores):
        b, h = core // 2, core % 2
        if h == 0:
            x_h, x_m = hsi[b], msi[b]
        else:
            x_h = np.concatenate([hsi[b][:, NQ:], hsi[b][:, :NQ]], axis=1)
            x_m = np.concatenate([msi[b][:, NQ:], msi[b][:, :NQ]], axis=1)
        m = {"x_h": np.ascontiguousarray(x_h), "x_m": np.ascontiguousarray(x_m)}
        m.update(weights)
        in_maps.append(m)
    return in_maps


def make_weight_map(
    wq1, bq1, wk1, bk1, wv1, bv1, wq2, bq2, wk2, bk2, wv2, bv2, wg, bg, wp, bp
):
    f = np.float32
    col = lambda v: np.ascontiguousarray(np.asarray(v, f).reshape(C, 1))
    tr = lambda w: np.ascontiguousarray(np.asarray(w, f).T)
    return {
        "wq1T": tr(wq1), "wk1T": tr(wk1), "wv1T": tr(wv1),
        "wq2T": tr(wq2), "wk2T": tr(wk2), "wv2T": tr(wv2),
        "wgaT": tr(np.asarray(wg, f)[:, :C]),
        "wgbT": tr(np.asarray(wg, f)[:, C:]),
        "wpTs": tr(0.5 * np.asarray(wp, f)),
        "bq1": col(bq1), "bk1": col(bk1), "bq2": col(bq2), "bk2": col(bk2),
        "bv1": col(bv1), "bv2": col(bv2), "bgh": col(0.5 * np.asarray(bg, f)),
        "bp": col(bp),
        "ones": np.ones((C, C), np.float32),
    }


_NC_CACHE = {}


def _get_program():
    if "nc" not in _NC_CACHE:
        _NC_CACHE["nc"] = build_program()
    return _NC_CACHE["nc"]


def run_on_cores(in_maps, trace=False, **kwargs):
    from concourse.bass_utils import run_bass_kernel_spmd

    nc = _get_program()
    return run_bass_kernel_spmd(
        nc, in_maps, core_ids=list(range(len(in_maps))), trace=trace, **kwargs
    )


def kernel(
    hsi, msi, wq1, bq1, wk1, bk1, wv1, bv1, wq2, bq2, wk2, bk2, wv2, bv2,
    wg, bg, wp, bp,
):
    B, _, H, W = hsi.shape
    weights = make_weight_map(
        wq1, bq1, wk1, bk1, wv1, bv1, wq2, bq2, wk2, bk2, wv2, bv2, wg, bg, wp, bp
    )
    in_maps = make_in_maps(np.asarray(hsi), np.asarray(msi), weights)
    res = run_on_cores(in_maps)
    out = np.zeros((B, C, N_TOK), dtype=np.float32)
    for core in range(8):
        b, h = core // 2, core % 2
        out[b][:, h * NQ : (h + 1) * NQ] = res.results[core]["out"]
    return out.reshape(B, C, H, W)


# revision 7
# speedup vs baseline: 1.3442x; 1.0877x over previous
"""CrossAttentionFusion Trainium2 kernel.

Problem (per batch element b of 4, C=128 channels, N=4096 tokens):
    Q1 = wq1@hsi+bq1; K1 = wk1@msi+bk1; V1 = wv1@msi+bv1   (1x1 convs)
    Q2 = wq2@msi+bq2; K2 = wk2@hsi+bk2; V2 = wv2@hsi+bv2
    out1 = attn(Q1,K1,V1); out2 = attn(Q2,K2,V2)           (softmax over keys)
    g = sigmoid(wg@[hsi;msi]+bg)
    out = wp@(g*out1 + (1-g)*out2) + bp

Sharding: 8 cores = (b, query-half). Each core computes 2048 query columns
for one batch element; keys/values span all 4096 tokens. Host permutes the
token axis per core so its queries are the first 2048 columns (key order is
irrelevant to attention sums), so the SPMD program is offset-free.

Core dataflow (transposed attention, keys on partitions):
    sT[m,n] = K[:,m]^T Q[:,n]        via matmul(lhsT=K tile, rhs=Q)
    pT = exp(scale*sT)               ACT, direct from PSUM
    den[n] = sum_m pT[m,n]           via matmul(lhsT=ones[128,128]) -> bcast rows
    outU[c,n] = sum_m VT[m,c] pT[m,n] accumulated over key tiles in PSUM
    out = outU * (1/den) + bv        (V-bias folds through softmax exactly)
Gate uses sigmoid(z) = 0.5*tanh(0.5 z)+0.5; the 0.5 factors fold into wp.
Matmuls run as float32r (full-rate fp32); the V-producing convs stay
float32 (exact) since their N=128 free dim gets no fp32r speedup anyway.
"""

import sys

if "/opt/trn_rl_repo" not in sys.path:
    sys.path.insert(0, "/opt/trn_rl_repo")

from contextlib import ExitStack

import numpy as np

import concourse.bacc as bacc
import concourse.bass as bass  # noqa: F401
import concourse.tile as tile
from concourse import mybir

F32 = mybir.dt.float32
F32R = mybir.dt.float32r
C = 128
N_TOK = 4096
NQ = 2048
FD = 512  # matmul moving-operand max for 4-byte dtypes
CH = 1024  # query-chunk width (PSUM accumulator width)
SCALE = 1.0 / float(np.sqrt(np.float32(C)))

WEIGHT_NAMES = ["wq1T", "wk1T", "wv1T", "wq2T", "wk2T", "wv2T", "wgaT", "wgbT", "wpTs"]
BIAS_NAMES = ["bq1", "bk1", "bq2", "bk2", "bv1", "bv2", "bgh", "bp"]


def _r(ap):
    return ap.bitcast(F32R)


def build_program(n_tok=N_TOK, nq=NQ, ch=CH, fd=FD):
    mt = n_tok // 128  # key tiles
    nch = nq // ch  # query chunks
    spc = ch // fd  # matmul slices per chunk
    vtg = ch // 128  # VT tiles per eviction group

    nc = bacc.Bacc("TRN2", target_bir_lowering=False, debug=False)
    din = {}
    for name in ["x_h", "x_m"]:
        din[name] = nc.dram_tensor(name, [C, n_tok], F32, kind="ExternalInput").ap()
    nw = len(WEIGHT_NAMES) + 1  # +1 for the all-ones block
    din["wpack"] = nc.dram_tensor("wpack", [C, nw * C], F32, kind="ExternalInput").ap()
    din["bpack"] = nc.dram_tensor(
        "bpack", [C, len(BIAS_NAMES)], F32, kind="ExternalInput"
    ).ap()
    out_d = nc.dram_tensor("out", [C, nq], F32, kind="ExternalOutput").ap()

    with ExitStack() as ctx:
        tc = ctx.enter_context(tile.TileContext(nc))
        const = ctx.enter_context(tc.tile_pool(name="const", bufs=1))
        big = ctx.enter_context(tc.tile_pool(name="big", bufs=1))
        ppool = ctx.enter_context(tc.tile_pool(name="ppool", bufs=4))
        rpool = ctx.enter_context(tc.tile_pool(name="rpool", bufs=2))
        stpool = ctx.enter_context(tc.tile_pool(name="stpool", bufs=2))
        ps_pool = ctx.enter_context(tc.tile_pool(name="ps", bufs=2, space="PSUM"))
        pacc_pool = ctx.enter_context(tc.tile_pool(name="pacc", bufs=2, space="PSUM"))

        # constants in: one packed DMA for weights (gpsimd SWDGE ring),
        # one for biases, so the head isn't serialized on per-tensor DMAs
        wpack_sb = const.tile([C, nw * C], F32R, name="wpack")
        nc.gpsimd.dma_start(out=wpack_sb[:], in_=_r(din["wpack"][:]))
        bpack_sb = const.tile([C, len(BIAS_NAMES)], F32, name="bpack")
        nc.gpsimd.dma_start(out=bpack_sb[:], in_=din["bpack"][:])
        w_sb = {
            name: wpack_sb[:, i * C : (i + 1) * C]
            for i, name in enumerate(WEIGHT_NAMES)
        }
        ones_sb = wpack_sb[:, len(WEIGHT_NAMES) * C :]
        b_sb = {name: bpack_sb[:, i : i + 1] for i, name in enumerate(BIAS_NAMES)}

        # activations in, chunked so convs start early; the two inputs go to
        # the two independent HWDGE rings (SP + ACT) to halve the head time
        xh_sb = big.tile([C, n_tok], F32R, name="xh")
        xm_sb = big.tile([C, n_tok], F32R, name="xm")
        for j in range(n_tok // fd):
            sl = slice(j * fd, (j + 1) * fd)
            nc.scalar.dma_start(out=xh_sb[:, sl], in_=_r(din["x_h"][:, sl]))
            nc.sync.dma_start(out=xm_sb[:, sl], in_=_r(din["x_m"][:, sl]))

        K1_sb = big.tile([C, n_tok], F32R, name="K1")
        K2_sb = big.tile([C, n_tok], F32R, name="K2")
        VT1_sb = big.tile([C, n_tok], F32R, name="VT1")
        VT2_sb = big.tile([C, n_tok], F32R, name="VT2")
        Q1_sb = big.tile([C, nq], F32R, name="Q1")
        Q2_sb = big.tile([C, nq], F32R, name="Q2")
        o1_sb = big.tile([C, nq], F32R, name="o1")
        o2_sb = big.tile([C, nq], F32R, name="o2")
        t_sb = big.tile([C, nq], F32R, name="t")
        d_sb = big.tile([C, nq], F32R, name="d")

        def conv(dst_sb, wT_sb, x_sb, cols, bias_sb):
            # dst[:, :cols] = wT.T @ x[:, :cols] (+ bias per channel)
            for j in range(cols // ch):
                ps = ps_pool.tile([C, ch], F32, tag="ps")
                for s in range(spc):
                    sl = slice(j * ch + s * fd, j * ch + (s + 1) * fd)
                    nc.tensor.matmul(
                        ps[:, s * fd : (s + 1) * fd],
                        wT_sb,
                        x_sb[:, sl],
                        start=True,
                        stop=True,
                    )
                dsl = slice(j * ch, (j + 1) * ch)
                if bias_sb is None:
                    nc.vector.tensor_copy(dst_sb[:, dsl], ps[:])
                else:
                    nc.vector.tensor_scalar_add(dst_sb[:, dsl], ps[:], bias_sb)

        def vt_conv(dst_sb, x_sb, wvT_sb):
            # dst tile j holds V^T rows for tokens [128j, 128j+128): [tok, chan]
            for g in range(mt // vtg):
                ps = ps_pool.tile([C, ch], F32, tag="ps")
                for u in range(vtg):
                    j = g * vtg + u
                    nc.tensor.matmul(
                        ps[:, u * 128 : (u + 1) * 128],
                        x_sb[:, j * 128 : (j + 1) * 128],
                        wvT_sb,
                        start=True,
                        stop=True,
                    )
                nc.scalar.copy(dst_sb[:, g * ch : (g + 1) * ch], ps[:])

        conv(K1_sb, w_sb["wk1T"], xm_sb, n_tok, b_sb["bk1"])
        conv(Q1_sb, w_sb["wq1T"], xh_sb, nq, b_sb["bq1"])
        vt_conv(VT1_sb, xm_sb, w_sb["wv1T"])
        conv(K2_sb, w_sb["wk2T"], xh_sb, n_tok, b_sb["bk2"])
        conv(Q2_sb, w_sb["wq2T"], xm_sb, nq, b_sb["bq2"])
        vt_conv(VT2_sb, xh_sb, w_sb["wv2T"])

        # gate: t = tanh(0.5*(wgA@xq_h + wgB@xq_m) + 0.5*bg)
        for j in range(nq // ch):
            ps = ps_pool.tile([C, ch], F32, tag="ps")
            for s in range(spc):
                sl = slice(j * ch + s * fd, j * ch + (s + 1) * fd)
                psl = ps[:, s * fd : (s + 1) * fd]
                nc.tensor.matmul(
                    psl, w_sb["wgaT"], xh_sb[:, sl], start=True, stop=False
                )
                nc.tensor.matmul(
                    psl, w_sb["wgbT"], xm_sb[:, sl], start=False, stop=True
                )
            nc.scalar.activation(
                t_sb[:, j * ch : (j + 1) * ch],
                ps[:],
                mybir.ActivationFunctionType.Tanh,
                bias=b_sb["bgh"],
                scale=0.5,
            )

        def attention(o_sb, Q_sb, K_sb, VT_sb, bv_sb, chunk_post=None):
            # Software-pipelined: scores for key-tile m+1 are emitted before
            # the den/PV matmuls of tile m, so the PE never sits waiting on
            # ACT's exp of tile m (exp overlaps the next 6 matmuls).
            pending = [None]
            for cidx in range(nch):
                p_out = pacc_pool.tile([C, ch], F32, tag="acc")
                p_den = pacc_pool.tile([C, ch], F32, tag="acc")
                pts = {}

                def scores(m):
                    ksl = slice(m * 128, (m + 1) * 128)
                    ps = ps_pool.tile([C, ch], F32, tag="ps")
                    for s in range(spc):
                        qsl = slice(cidx * ch + s * fd, cidx * ch + (s + 1) * fd)
                        nc.tensor.matmul(
                            ps[:, s * fd : (s + 1) * fd],
                            K_sb[:, ksl],
                            Q_sb[:, qsl],
                            start=True,
                            stop=True,
                        )
                    pt = ppool.tile([C, ch], F32R, tag="pt")
                    nc.scalar.activation(
                        pt[:], ps[:], mybir.ActivationFunctionType.Exp, scale=SCALE
                    )
                    pts[m] = pt

                def accum(m):
                    ksl = slice(m * 128, (m + 1) * 128)
                    pt = pts.pop(m)
                    first, last = m == 0, m == mt - 1
                    for s in range(spc):
                        ssl = slice(s * fd, (s + 1) * fd)
                        nc.tensor.matmul(
                            p_den[:, ssl], ones_sb, pt[:, ssl],
                            start=first, stop=last,
                        )
                        nc.tensor.matmul(
                            p_out[:, ssl], VT_sb[:, ksl], pt[:, ssl],
                            start=first, stop=last,
                        )

                scores(0)
                for m in range(1, mt):
                    scores(m)
                    accum(m - 1)
                    if m == min(4, mt - 1) and pending[0] is not None:
                        pending[0]()
                        pending[0] = None
                accum(mt - 1)

                rec = rpool.tile([C, ch], F32, tag="rec")
                nc.vector.reciprocal_approx_fast(rec[:], p_den[:])
                osl = slice(cidx * ch, (cidx + 1) * ch)
                nc.vector.tensor_mul(o_sb[:, osl], p_out[:], rec[:])
                nc.vector.tensor_scalar_add(o_sb[:, osl], o_sb[:, osl], bv_sb)
                if chunk_post is not None:
                    if pending[0] is not None:
                        pending[0]()
                    pending[0] = lambda c=cidx: chunk_post(c)
            if pending[0] is not None:
                pending[0]()
                pending[0] = None

        attention(o1_sb, Q1_sb, K1_sb, VT1_sb, b_sb["bv1"])

        def fuse_and_project(cidx):
            # fused' = (o1+o2) + t*(o1-o2);  out = (0.5*wp)^T.T @ fused' + bp
            sl = slice(cidx * ch, (cidx + 1) * ch)
            nc.vector.tensor_sub(d_sb[:, sl], o1_sb[:, sl], o2_sb[:, sl])
            nc.vector.tensor_add(o1_sb[:, sl], o1_sb[:, sl], o2_sb[:, sl])
            nc.vector.tensor_mul(d_sb[:, sl], d_sb[:, sl], t_sb[:, sl])
            nc.vector.tensor_add(o1_sb[:, sl], o1_sb[:, sl], d_sb[:, sl])
            for s in range(spc):
                psl = slice(cidx * ch + s * fd, cidx * ch + (s + 1) * fd)
                ps = ps_pool.tile([C, ch], F32, tag="ps")
                nc.tensor.matmul(
                    ps[:, :fd], w_sb["wpTs"], o1_sb[:, psl], start=True, stop=True
                )
                st = stpool.tile([C, fd], F32, tag="st")
                nc.vector.tensor_scalar_add(st[:], ps[:, :fd], b_sb["bp"])
                nc.sync.dma_start(out=out_d[:, psl], in_=st[:])

        attention(o2_sb, Q2_sb, K2_sb, VT2_sb, b_sb["bv2"], chunk_post=fuse_and_project)

    nc.compile()
    return nc


def make_in_maps(hsi, msi, weights, n_cores=8):
    """Host-side sharding: core i handles (b=i//2, half=i%2); the token axis is
    rotated so the core's queries are columns [0, NQ)."""
    B = hsi.shape[0]
    hsi = np.ascontiguousarray(hsi.reshape(B, C, N_TOK), dtype=np.float32)
    msi = np.ascontiguousarray(msi.reshape(B, C, N_TOK), dtype=np.float32)
    in_maps = []
    for core in range(n_cores):
        b, h = core // 2, core % 2
        if h == 0:
            x_h, x_m = hsi[b], msi[b]
        else:
            x_h = np.concatenate([hsi[b][:, NQ:], hsi[b][:, :NQ]], axis=1)
            x_m = np.concatenate([msi[b][:, NQ:], msi[b][:, :NQ]], axis=1)
        m = {"x_h": np.ascontiguousarray(x_h), "x_m": np.ascontiguousarray(x_m)}
        m.update(weights)
        in_maps.append(m)
    return in_maps


def make_weight_map(
    wq1, bq1, wk1, bk1, wv1, bv1, wq2, bq2, wk2, bk2, wv2, bv2, wg, bg, wp, bp
):
    f = np.float32
    col = lambda v: np.ascontiguousarray(np.asarray(v, f).reshape(C, 1))
    tr = lambda w: np.ascontiguousarray(np.asarray(w, f).T)
    w = {
        "wq1T": tr(wq1), "wk1T": tr(wk1), "wv1T": tr(wv1),
        "wq2T": tr(wq2), "wk2T": tr(wk2), "wv2T": tr(wv2),
        "wgaT": tr(np.asarray(wg, f)[:, :C]),
        "wgbT": tr(np.asarray(wg, f)[:, C:]),
        "wpTs": tr(0.5 * np.asarray(wp, f)),
    }
    b = {
        "bq1": col(bq1), "bk1": col(bk1), "bq2": col(bq2), "bk2": col(bk2),
        "bv1": col(bv1), "bv2": col(bv2), "bgh": col(0.5 * np.asarray(bg, f)),
        "bp": col(bp),
    }
    wpack = np.concatenate(
        [w[n] for n in WEIGHT_NAMES] + [np.ones((C, C), f)], axis=1
    )
    bpack = np.concatenate([b[n] for n in BIAS_NAMES], axis=1)
    return {
        "wpack": np.ascontiguousarray(wpack),
        "bpack": np.ascontiguousarray(bpack),
    }


_NC_CACHE = {}


def _get_program():
    if "nc" not in _NC_CACHE:
        _NC_CACHE["nc"] = build_program()
    return _NC_CACHE["nc"]


def run_on_cores(in_maps, trace=False, **kwargs):
    from concourse.bass_utils import run_bass_kernel_spmd

    nc = _get_program()
    return run_bass_kernel_spmd(
        nc, in_maps, core_ids=list(range(len(in_maps))), trace=trace, **kwargs
    )


def kernel(
    hsi, msi, wq1, bq1, wk1, bk1, wv1, bv1, wq2, bq2, wk2, bk2, wv2, bv2,
    wg, bg, wp, bp,
):
    B, _, H, W = hsi.shape
    weights = make_weight_map(
        wq1, bq1, wk1, bk1, wv1, bv1, wq2, bq2, wk2, bk2, wv2, bv2, wg, bg, wp, bp
    )
    in_maps = make_in_maps(np.asarray(hsi), np.asarray(msi), weights)
    res = run_on_cores(in_maps)
    out = np.zeros((B, C, N_TOK), dtype=np.float32)
    for core in range(8):
        b, h = core // 2, core % 2
        out[b][:, h * NQ : (h + 1) * NQ] = res.results[core]["out"]
    return out.reshape(B, C, H, W)


# revision 8
# speedup vs baseline: 1.3989x; 1.0407x over previous
"""CrossAttentionFusion Trainium2 kernel.

Problem (per batch element b of 4, C=128 channels, N=4096 tokens):
    Q1 = wq1@hsi+bq1; K1 = wk1@msi+bk1; V1 = wv1@msi+bv1   (1x1 convs)
    Q2 = wq2@msi+bq2; K2 = wk2@hsi+bk2; V2 = wv2@hsi+bv2
    out1 = attn(Q1,K1,V1); out2 = attn(Q2,K2,V2)           (softmax over keys)
    g = sigmoid(wg@[hsi;msi]+bg)
    out = wp@(g*out1 + (1-g)*out2) + bp

Sharding: 8 cores = (b, query-half). Each core computes 2048 query columns
for one batch element; keys/values span all 4096 tokens. Host permutes the
token axis per core so its queries are the first 2048 columns (key order is
irrelevant to attention sums), so the SPMD program is offset-free.

Core dataflow (transposed attention, keys on partitions):
    sT[m,n] = K[:,m]^T Q[:,n]        via matmul(lhsT=K tile, rhs=Q)
    pT = exp(scale*sT)               ACT, direct from PSUM
    den[n] = sum_m pT[m,n]           via matmul(lhsT=ones[128,128]) -> bcast rows
    outU[c,n] = sum_m VT[m,c] pT[m,n] accumulated over key tiles in PSUM
    out = outU * (1/den) + bv        (V-bias folds through softmax exactly)
Gate uses sigmoid(z) = 0.5*tanh(0.5 z)+0.5; the 0.5 factors fold into wp.
Matmuls run as float32r (full-rate fp32); the V-producing convs stay
float32 (exact) since their N=128 free dim gets no fp32r speedup anyway.
"""

import sys

if "/opt/trn_rl_repo" not in sys.path:
    sys.path.insert(0, "/opt/trn_rl_repo")

from contextlib import ExitStack

import numpy as np

import concourse.bacc as bacc
import concourse.bass as bass  # noqa: F401
import concourse.tile as tile
from concourse import mybir

F32 = mybir.dt.float32
F32R = mybir.dt.float32r
C = 128
N_TOK = 4096
NQ = 2048
FD = 512  # matmul moving-operand max for 4-byte dtypes
CH = 1024  # query-chunk width (PSUM accumulator width)
SCALE = 1.0 / float(np.sqrt(np.float32(C)))

WEIGHT_NAMES = ["wq1T", "wk1T", "wv1T", "wq2T", "wk2T", "wv2T", "wgaT", "wgbT", "wpTs"]
BIAS_NAMES = ["bq1", "bk1", "bq2", "bk2", "bv1", "bv2", "bgh", "bp"]


def _r(ap):
    return ap.bitcast(F32R)


def build_program(n_tok=N_TOK, nq=NQ, ch=CH, fd=FD):
    mt = n_tok // 128  # key tiles
    nch = nq // ch  # query chunks
    spc = ch // fd  # matmul slices per chunk
    vtg = ch // 128  # VT tiles per eviction group

    nc = bacc.Bacc("TRN2", target_bir_lowering=False, debug=False)
    din = {}
    for name in ["x_h", "x_m"]:
        din[name] = nc.dram_tensor(name, [C, n_tok], F32, kind="ExternalInput").ap()
    nw = len(WEIGHT_NAMES) + 1  # +1 for the all-ones block
    din["wpack"] = nc.dram_tensor("wpack", [C, nw * C], F32, kind="ExternalInput").ap()
    din["bpack"] = nc.dram_tensor(
        "bpack", [C, len(BIAS_NAMES)], F32, kind="ExternalInput"
    ).ap()
    out_d = nc.dram_tensor("out", [C, nq], F32, kind="ExternalOutput").ap()

    with ExitStack() as ctx:
        tc = ctx.enter_context(tile.TileContext(nc))
        const = ctx.enter_context(tc.tile_pool(name="const", bufs=1))
        big = ctx.enter_context(tc.tile_pool(name="big", bufs=1))
        ppool = ctx.enter_context(tc.tile_pool(name="ppool", bufs=4))
        rpool = ctx.enter_context(tc.tile_pool(name="rpool", bufs=1))
        sppool = ctx.enter_context(tc.tile_pool(name="sppool", bufs=2))
        stpool = ctx.enter_context(tc.tile_pool(name="stpool", bufs=2))
        ps_pool = ctx.enter_context(tc.tile_pool(name="ps", bufs=2, space="PSUM"))
        pacc_pool = ctx.enter_context(tc.tile_pool(name="pacc", bufs=2, space="PSUM"))

        # constants in: one packed DMA for weights (gpsimd SWDGE ring),
        # one for biases, so the head isn't serialized on per-tensor DMAs
        wpack_sb = const.tile([C, nw * C], F32R, name="wpack")
        nc.gpsimd.dma_start(out=wpack_sb[:], in_=_r(din["wpack"][:]))
        bpack_sb = const.tile([C, len(BIAS_NAMES)], F32, name="bpack")
        nc.gpsimd.dma_start(out=bpack_sb[:], in_=din["bpack"][:])
        w_sb = {
            name: wpack_sb[:, i * C : (i + 1) * C]
            for i, name in enumerate(WEIGHT_NAMES)
        }
        ones_sb = wpack_sb[:, len(WEIGHT_NAMES) * C :]
        b_sb = {name: bpack_sb[:, i : i + 1] for i, name in enumerate(BIAS_NAMES)}

        # activations in, chunked so convs start early; the two inputs go to
        # the two independent HWDGE rings (SP + ACT) to halve the head time
        xh_sb = big.tile([C, n_tok], F32R, name="xh")
        xm_sb = big.tile([C, n_tok], F32R, name="xm")
        for j in range(n_tok // fd):
            sl = slice(j * fd, (j + 1) * fd)
            nc.scalar.dma_start(out=xh_sb[:, sl], in_=_r(din["x_h"][:, sl]))
            nc.sync.dma_start(out=xm_sb[:, sl], in_=_r(din["x_m"][:, sl]))

        K1_sb = big.tile([C, n_tok], F32R, name="K1")
        K2_sb = big.tile([C, n_tok], F32R, name="K2")
        VT1_sb = big.tile([C, n_tok], F32R, name="VT1")
        VT2_sb = big.tile([C, n_tok], F32R, name="VT2")
        Q1_sb = big.tile([C, nq], F32R, name="Q1")
        Q2_sb = big.tile([C, nq], F32R, name="Q2")
        o1_sb = big.tile([C, nq], F32R, name="o1")
        o2_sb = big.tile([C, nq], F32R, name="o2")
        t_sb = big.tile([C, nq], F32R, name="t")
        d_sb = big.tile([C, nq], F32R, name="d")

        def conv(dst_sb, wT_sb, x_sb, cols, bias_sb):
            # dst[:, :cols] = wT.T @ x[:, :cols] (+ bias per channel)
            for j in range(cols // ch):
                ps = ps_pool.tile([C, ch], F32, tag="ps")
                for s in range(spc):
                    sl = slice(j * ch + s * fd, j * ch + (s + 1) * fd)
                    nc.tensor.matmul(
                        ps[:, s * fd : (s + 1) * fd],
                        wT_sb,
                        x_sb[:, sl],
                        start=True,
                        stop=True,
                    )
                dsl = slice(j * ch, (j + 1) * ch)
                if bias_sb is None:
                    nc.vector.tensor_copy(dst_sb[:, dsl], ps[:])
                else:
                    nc.vector.tensor_scalar_add(dst_sb[:, dsl], ps[:], bias_sb)

        def vt_conv(dst_sb, x_sb, wvT_sb):
            # dst tile j holds V^T rows for tokens [128j, 128j+128): [tok, chan]
            for g in range(mt // vtg):
                ps = ps_pool.tile([C, ch], F32, tag="ps")
                for u in range(vtg):
                    j = g * vtg + u
                    nc.tensor.matmul(
                        ps[:, u * 128 : (u + 1) * 128],
                        x_sb[:, j * 128 : (j + 1) * 128],
                        wvT_sb,
                        start=True,
                        stop=True,
                    )
                nc.scalar.copy(dst_sb[:, g * ch : (g + 1) * ch], ps[:])

        conv(K1_sb, w_sb["wk1T"], xm_sb, n_tok, b_sb["bk1"])
        conv(Q1_sb, w_sb["wq1T"], xh_sb, nq, b_sb["bq1"])
        vt_conv(VT1_sb, xm_sb, w_sb["wv1T"])
        conv(K2_sb, w_sb["wk2T"], xh_sb, n_tok, b_sb["bk2"])
        conv(Q2_sb, w_sb["wq2T"], xm_sb, nq, b_sb["bq2"])
        vt_conv(VT2_sb, xh_sb, w_sb["wv2T"])

        # gate: t = tanh(0.5*(wgA@xq_h + wgB@xq_m) + 0.5*bg)
        for j in range(nq // ch):
            ps = ps_pool.tile([C, ch], F32, tag="ps")
            for s in range(spc):
                sl = slice(j * ch + s * fd, j * ch + (s + 1) * fd)
                psl = ps[:, s * fd : (s + 1) * fd]
                nc.tensor.matmul(
                    psl, w_sb["wgaT"], xh_sb[:, sl], start=True, stop=False
                )
                nc.tensor.matmul(
                    psl, w_sb["wgbT"], xm_sb[:, sl], start=False, stop=True
                )
            nc.scalar.activation(
                t_sb[:, j * ch : (j + 1) * ch],
                ps[:],
                mybir.ActivationFunctionType.Tanh,
                bias=b_sb["bgh"],
                scale=0.5,
            )

        def attention(o_sb, Q_sb, K_sb, VT_sb, bv_sb, chunk_post=None):
            # Software-pipelined: scores for key-tile m+1 are emitted before
            # the den/PV matmuls of tile m, so the PE never sits waiting on
            # ACT's exp of tile m (exp overlaps the next 6 matmuls).
            pending = [None]
            for cidx in range(nch):
                p_out = pacc_pool.tile([C, ch], F32, tag="acc")
                p_den = pacc_pool.tile([C, ch], F32, tag="acc")
                pts = {}

                def scores(m):
                    ksl = slice(m * 128, (m + 1) * 128)
                    ps = ps_pool.tile([C, ch], F32, tag="ps")
                    for s in range(spc):
                        qsl = slice(cidx * ch + s * fd, cidx * ch + (s + 1) * fd)
                        nc.tensor.matmul(
                            ps[:, s * fd : (s + 1) * fd],
                            K_sb[:, ksl],
                            Q_sb[:, qsl],
                            start=True,
                            stop=True,
                        )
                    pt = ppool.tile([C, ch], F32R, tag="pt")
                    nc.scalar.activation(
                        pt[:], ps[:], mybir.ActivationFunctionType.Exp, scale=SCALE
                    )
                    pts[m] = pt

                def accum(m):
                    # PV accumulation only; the softmax denominator is fed by
                    # pair() below (DVE pair-sums halve the ones-matmul count)
                    ksl = slice(m * 128, (m + 1) * 128)
                    pt = pts[m]
                    first, last = m == 0, m == mt - 1
                    for s in range(spc):
                        ssl = slice(s * fd, (s + 1) * fd)
                        nc.tensor.matmul(
                            p_out[:, ssl], VT_sb[:, ksl], pt[:, ssl],
                            start=first, stop=last,
                        )

                def pair(k):
                    pa, pb = pts.pop(2 * k), pts.pop(2 * k + 1)
                    sp = sppool.tile([C, ch], F32R, tag="sp")
                    nc.vector.tensor_add(sp[:], pa[:], pb[:])
                    first, last = k == 0, k == mt // 2 - 1
                    for s in range(spc):
                        ssl = slice(s * fd, (s + 1) * fd)
                        nc.tensor.matmul(
                            p_den[:, ssl], ones_sb, sp[:, ssl],
                            start=first, stop=last,
                        )

                scores(0)
                for m in range(1, mt):
                    scores(m)
                    accum(m - 1)
                    if m >= 2 and m % 2 == 0:
                        pair((m - 2) // 2)
                    if m == min(4, mt - 1) and pending[0] is not None:
                        pending[0]()
                        pending[0] = None
                accum(mt - 1)
                pair(mt // 2 - 1)

                rec = rpool.tile([C, ch], F32, tag="rec")
                nc.vector.reciprocal_approx_fast(rec[:], p_den[:])
                osl = slice(cidx * ch, (cidx + 1) * ch)
                nc.vector.tensor_mul(o_sb[:, osl], p_out[:], rec[:])
                nc.vector.tensor_scalar_add(o_sb[:, osl], o_sb[:, osl], bv_sb)
                if chunk_post is not None:
                    if pending[0] is not None:
                        pending[0]()
                    pending[0] = lambda c=cidx: chunk_post(c)
            if pending[0] is not None:
                pending[0]()
                pending[0] = None

        attention(o1_sb, Q1_sb, K1_sb, VT1_sb, b_sb["bv1"])

        def fuse_and_project(cidx):
            # fused' = (o1+o2) + t*(o1-o2);  out = (0.5*wp)^T.T @ fused' + bp
            sl = slice(cidx * ch, (cidx + 1) * ch)
            nc.vector.tensor_sub(d_sb[:, sl], o1_sb[:, sl], o2_sb[:, sl])
            nc.vector.tensor_add(o1_sb[:, sl], o1_sb[:, sl], o2_sb[:, sl])
            nc.vector.tensor_mul(d_sb[:, sl], d_sb[:, sl], t_sb[:, sl])
            nc.vector.tensor_add(o1_sb[:, sl], o1_sb[:, sl], d_sb[:, sl])
            for s in range(spc):
                psl = slice(cidx * ch + s * fd, cidx * ch + (s + 1) * fd)
                ps = ps_pool.tile([C, ch], F32, tag="ps")
                nc.tensor.matmul(
                    ps[:, :fd], w_sb["wpTs"], o1_sb[:, psl], start=True, stop=True
                )
                st = stpool.tile([C, fd], F32, tag="st")
                nc.vector.tensor_scalar_add(st[:], ps[:, :fd], b_sb["bp"])
                nc.sync.dma_start(out=out_d[:, psl], in_=st[:])

        attention(o2_sb, Q2_sb, K2_sb, VT2_sb, b_sb["bv2"], chunk_post=fuse_and_project)

    nc.compile()
    return nc


def make_in_maps(hsi, msi, weights, n_cores=8):
    """Host-side sharding: core i handles (b=i//2, half=i%2); the token axis is
    rotated so the core's queries are columns [0, NQ)."""
    B = hsi.shape[0]
    hsi = np.ascontiguousarray(hsi.reshape(B, C, N_TOK), dtype=np.float32)
    msi = np.ascontiguousarray(msi.reshape(B, C, N_TOK), dtype=np.float32)
    in_maps = []
    for core in range(n_cores):
        b, h = core // 2, core % 2
        if h == 0:
            x_h, x_m = hsi[b], msi[b]
        else:
            x_h = np.concatenate([hsi[b][:, NQ:], hsi[b][:, :NQ]], axis=1)
            x_m = np.concatenate([msi[b][:, NQ:], msi[b][:, :NQ]], axis=1)
        m = {"x_h": np.ascontiguousarray(x_h), "x_m": np.ascontiguousarray(x_m)}
        m.update(weights)
        in_maps.append(m)
    return in_maps


def make_weight_map(
    wq1, bq1, wk1, bk1, wv1, bv1, wq2, bq2, wk2, bk2, wv2, bv2, wg, bg, wp, bp
):
    f = np.float32
    col = lambda v: np.ascontiguousarray(np.asarray(v, f).reshape(C, 1))
    tr = lambda w: np.ascontiguousarray(np.asarray(w, f).T)
    w = {
        "wq1T": tr(wq1), "wk1T": tr(wk1), "wv1T": tr(wv1),
        "wq2T": tr(wq2), "wk2T": tr(wk2), "wv2T": tr(wv2),
        "wgaT": tr(np.asarray(wg, f)[:, :C]),
        "wgbT": tr(np.asarray(wg, f)[:, C:]),
        "wpTs": tr(0.5 * np.asarray(wp, f)),
    }
    b = {
        "bq1": col(bq1), "bk1": col(bk1), "bq2": col(bq2), "bk2": col(bk2),
        "bv1": col(bv1), "bv2": col(bv2), "bgh": col(0.5 * np.asarray(bg, f)),
        "bp": col(bp),
    }
    wpack = np.concatenate(
        [w[n] for n in WEIGHT_NAMES] + [np.ones((C, C), f)], axis=1
    )
    bpack = np.concatenate([b[n] for n in BIAS_NAMES], axis=1)
    return {
        "wpack": np.ascontiguousarray(wpack),
        "bpack": np.ascontiguousarray(bpack),
    }


_NC_CACHE = {}


def _get_program():
    if "nc" not in _NC_CACHE:
        _NC_CACHE["nc"] = build_program()
    return _NC_CACHE["nc"]


def run_on_cores(in_maps, trace=False, **kwargs):
    from concourse.bass_utils import run_bass_kernel_spmd

    nc = _get_program()
    return run_bass_kernel_spmd(
        nc, in_maps, core_ids=list(range(len(in_maps))), trace=trace, **kwargs
    )


def kernel(
    hsi, msi, wq1, bq1, wk1, bk1, wv1, bv1, wq2, bq2, wk2, bk2, wv2, bv2,
    wg, bg, wp, bp,
):
    B, _, H, W = hsi.shape
    weights = make_weight_map(
        wq1, bq1, wk1, bk1, wv1, bv1, wq2, bq2, wk2, bk2, wv2, bv2, wg, bg, wp, bp
    )
    in_maps = make_in_maps(np.asarray(hsi), np.asarray(msi), weights)
    res = run_on_cores(in_maps)
    out = np.zeros((B, C, N_TOK), dtype=np.float32)
    for core in range(8):
        b, h = core // 2, core % 2
        out[b][:, h * NQ : (h + 1) * NQ] = res.results[core]["out"]
    return out.reshape(B, C, H, W)


# revision 9
# speedup vs baseline: 1.4746x; 1.0541x over previous
"""CrossAttentionFusion Trainium2 kernel.

Problem (per batch element b of 4, C=128 channels, N=4096 tokens):
    Q1 = wq1@hsi+bq1; K1 = wk1@msi+bk1; V1 = wv1@msi+bv1   (1x1 convs)
    Q2 = wq2@msi+bq2; K2 = wk2@hsi+bk2; V2 = wv2@hsi+bv2
    out1 = attn(Q1,K1,V1); out2 = attn(Q2,K2,V2)           (softmax over keys)
    g = sigmoid(wg@[hsi;msi]+bg)
    out = wp@(g*out1 + (1-g)*out2) + bp

Sharding: 8 cores = (b, query-half). Each core computes 2048 query columns
for one batch element; keys/values span all 4096 tokens. Host permutes the
token axis per core so its queries are the first 2048 columns (key order is
irrelevant to attention sums), so the SPMD program is offset-free.

Core dataflow (transposed attention, keys on partitions):
    sT[m,n] = K[:,m]^T Q[:,n]        via matmul(lhsT=K tile, rhs=Q)
    pT = exp(scale*sT)               ACT, direct from PSUM
    den[n] = sum_m pT[m,n]           via matmul(lhsT=ones[128,128]) -> bcast rows
    outU[c,n] = sum_m VT[m,c] pT[m,n] accumulated over key tiles in PSUM
    out = outU * (1/den) + bv        (V-bias folds through softmax exactly)
Gate uses sigmoid(z) = 0.5*tanh(0.5 z)+0.5; the 0.5 factors fold into wp.
Matmuls run as float32r (full-rate fp32); the V-producing convs stay
float32 (exact) since their N=128 free dim gets no fp32r speedup anyway.
"""

import sys

if "/opt/trn_rl_repo" not in sys.path:
    sys.path.insert(0, "/opt/trn_rl_repo")

from contextlib import ExitStack

import numpy as np

import concourse.bacc as bacc
import concourse.bass as bass  # noqa: F401
import concourse.tile as tile
from concourse import mybir

F32 = mybir.dt.float32
F32R = mybir.dt.float32r
C = 128
N_TOK = 4096
NQ = 2048
FD = 512  # matmul moving-operand max for 4-byte dtypes
CH = 1024  # query-chunk width (PSUM accumulator width)
SCALE = 1.0 / float(np.sqrt(np.float32(C)))

WEIGHT_NAMES = ["wq1T", "wk1T", "wv1T", "wq2T", "wk2T", "wv2T", "wgaT", "wgbT", "wpTs"]
BIAS_NAMES = ["bq1", "bk1", "bq2", "bk2", "bv1", "bv2", "bgh", "bp"]


def _r(ap):
    return ap.bitcast(F32R)


def build_program(n_tok=N_TOK, nq=NQ, ch=CH, fd=FD):
    mt = n_tok // 128  # key tiles
    nch = nq // ch  # query chunks
    spc = ch // fd  # matmul slices per chunk
    vtg = ch // 128  # VT tiles per eviction group

    nc = bacc.Bacc("TRN2", target_bir_lowering=False, debug=False)
    din = {}
    for name in ["x_h", "x_m"]:
        din[name] = nc.dram_tensor(name, [C, n_tok], F32, kind="ExternalInput").ap()
    nw = len(WEIGHT_NAMES) + 1  # +1 for the all-ones block
    din["wpack"] = nc.dram_tensor("wpack", [C, nw * C], F32, kind="ExternalInput").ap()
    din["bpack"] = nc.dram_tensor(
        "bpack", [C, len(BIAS_NAMES)], F32, kind="ExternalInput"
    ).ap()
    out_d = nc.dram_tensor("out", [C, nq], F32, kind="ExternalOutput").ap()

    with ExitStack() as ctx:
        tc = ctx.enter_context(tile.TileContext(nc))
        const = ctx.enter_context(tc.tile_pool(name="const", bufs=1))
        big = ctx.enter_context(tc.tile_pool(name="big", bufs=1))
        ppool = ctx.enter_context(tc.tile_pool(name="ppool", bufs=4))
        rpool = ctx.enter_context(tc.tile_pool(name="rpool", bufs=1))
        sppool = ctx.enter_context(tc.tile_pool(name="sppool", bufs=2))
        stpool = ctx.enter_context(tc.tile_pool(name="stpool", bufs=2))
        ps_pool = ctx.enter_context(tc.tile_pool(name="ps", bufs=2, space="PSUM"))
        pacc_pool = ctx.enter_context(tc.tile_pool(name="pacc", bufs=2, space="PSUM"))

        # constants in: one packed DMA for weights (gpsimd SWDGE ring),
        # one for biases, so the head isn't serialized on per-tensor DMAs
        wpack_sb = const.tile([C, nw * C], F32R, name="wpack")
        nc.gpsimd.dma_start(out=wpack_sb[:], in_=_r(din["wpack"][:]))
        bpack_sb = const.tile([C, len(BIAS_NAMES)], F32, name="bpack")
        nc.gpsimd.dma_start(out=bpack_sb[:], in_=din["bpack"][:])
        w_sb = {
            name: wpack_sb[:, i * C : (i + 1) * C]
            for i, name in enumerate(WEIGHT_NAMES)
        }
        ones_sb = wpack_sb[:, len(WEIGHT_NAMES) * C :]
        b_sb = {name: bpack_sb[:, i : i + 1] for i, name in enumerate(BIAS_NAMES)}

        # activations in, chunked so convs start early; the two inputs go to
        # the two independent HWDGE rings (SP + ACT) to halve the head time
        xh_sb = big.tile([C, n_tok], F32R, name="xh")
        xm_sb = big.tile([C, n_tok], F32R, name="xm")
        for j in range(n_tok // fd):
            sl = slice(j * fd, (j + 1) * fd)
            nc.scalar.dma_start(out=xh_sb[:, sl], in_=_r(din["x_h"][:, sl]))
            nc.sync.dma_start(out=xm_sb[:, sl], in_=_r(din["x_m"][:, sl]))

        K1_sb = big.tile([C, n_tok], F32R, name="K1")
        K2_sb = big.tile([C, n_tok], F32R, name="K2")
        VT1_sb = big.tile([C, n_tok], F32R, name="VT1")
        VT2_sb = big.tile([C, n_tok], F32R, name="VT2")
        Q1_sb = big.tile([C, nq], F32R, name="Q1")
        Q2_sb = big.tile([C, nq], F32R, name="Q2")
        o1_sb = big.tile([C, nq], F32R, name="o1")
        o2_sb = big.tile([C, nq], F32R, name="o2")
        t_sb = big.tile([C, nq], F32R, name="t")
        d_sb = big.tile([C, nq], F32R, name="d")

        def conv(dst_sb, wT_sb, x_sb, cols, bias_sb):
            # dst[:, :cols] = wT.T @ x[:, :cols] (+ bias per channel)
            for j in range(cols // ch):
                ps = ps_pool.tile([C, ch], F32, tag="ps")
                for s in range(spc):
                    sl = slice(j * ch + s * fd, j * ch + (s + 1) * fd)
                    nc.tensor.matmul(
                        ps[:, s * fd : (s + 1) * fd],
                        wT_sb,
                        x_sb[:, sl],
                        start=True,
                        stop=True,
                    )
                dsl = slice(j * ch, (j + 1) * ch)
                if bias_sb is None:
                    nc.vector.tensor_copy(dst_sb[:, dsl], ps[:])
                else:
                    nc.vector.tensor_scalar_add(dst_sb[:, dsl], ps[:], bias_sb)

        def vt_conv(dst_sb, x_sb, wvT_sb):
            # dst tile j holds V^T rows for tokens [128j, 128j+128): [tok, chan]
            for g in range(mt // vtg):
                ps = ps_pool.tile([C, ch], F32, tag="ps")
                for u in range(vtg):
                    j = g * vtg + u
                    nc.tensor.matmul(
                        ps[:, u * 128 : (u + 1) * 128],
                        x_sb[:, j * 128 : (j + 1) * 128],
                        wvT_sb,
                        start=True,
                        stop=True,
                    )
                nc.scalar.copy(dst_sb[:, g * ch : (g + 1) * ch], ps[:])

        conv(K1_sb, w_sb["wk1T"], xm_sb, n_tok, b_sb["bk1"])
        conv(Q1_sb, w_sb["wq1T"], xh_sb, nq, b_sb["bq1"])
        conv(K2_sb, w_sb["wk2T"], xh_sb, n_tok, b_sb["bk2"])
        conv(Q2_sb, w_sb["wq2T"], xm_sb, nq, b_sb["bq2"])

        # gate: t = tanh(0.5*(wgA@xq_h + wgB@xq_m) + 0.5*bg)
        for j in range(nq // ch):
            ps = ps_pool.tile([C, ch], F32, tag="ps")
            for s in range(spc):
                sl = slice(j * ch + s * fd, j * ch + (s + 1) * fd)
                psl = ps[:, s * fd : (s + 1) * fd]
                nc.tensor.matmul(
                    psl, w_sb["wgaT"], xh_sb[:, sl], start=True, stop=False
                )
                nc.tensor.matmul(
                    psl, w_sb["wgbT"], xm_sb[:, sl], start=False, stop=True
                )
            nc.scalar.activation(
                t_sb[:, j * ch : (j + 1) * ch],
                ps[:],
                mybir.ActivationFunctionType.Tanh,
                bias=b_sb["bgh"],
                scale=0.5,
            )
        vt_conv(VT1_sb, xm_sb, w_sb["wv1T"])
        vt_conv(VT2_sb, xh_sb, w_sb["wv2T"])

        # Deferred-work queue: thunks drained one per two m-iterations inside
        # the attention loops, so serial DVE chains (normalize bias, gated
        # fusion, projection) never stall the in-order PE queue.
        events = []

        def attention(o_sb, Q_sb, K_sb, VT_sb, bv_sb, post_chunk=None):
            # Software-pipelined: scores for key-tile m+1 are emitted before
            # the PV matmuls of tile m, so the PE never sits waiting on
            # ACT's exp of tile m (exp overlaps the next matmuls).
            for cidx in range(nch):
                p_out = pacc_pool.tile([C, ch], F32, tag="acc")
                p_den = pacc_pool.tile([C, ch], F32, tag="acc")
                pts = {}

                def scores(m):
                    ksl = slice(m * 128, (m + 1) * 128)
                    ps = ps_pool.tile([C, ch], F32, tag="ps")
                    for s in range(spc):
                        qsl = slice(cidx * ch + s * fd, cidx * ch + (s + 1) * fd)
                        nc.tensor.matmul(
                            ps[:, s * fd : (s + 1) * fd],
                            K_sb[:, ksl],
                            Q_sb[:, qsl],
                            start=True,
                            stop=True,
                        )
                    pt = ppool.tile([C, ch], F32R, tag="pt")
                    nc.scalar.activation(
                        pt[:], ps[:], mybir.ActivationFunctionType.Exp, scale=SCALE
                    )
                    pts[m] = pt

                def accum(m):
                    # PV accumulation only; the softmax denominator is fed by
                    # pair() below (DVE pair-sums halve the ones-matmul count)
                    ksl = slice(m * 128, (m + 1) * 128)
                    pt = pts[m]
                    first, last = m == 0, m == mt - 1
                    for s in range(spc):
                        ssl = slice(s * fd, (s + 1) * fd)
                        nc.tensor.matmul(
                            p_out[:, ssl], VT_sb[:, ksl], pt[:, ssl],
                            start=first, stop=last,
                        )

                def pair(k):
                    pa, pb = pts.pop(2 * k), pts.pop(2 * k + 1)
                    sp = sppool.tile([C, ch], F32R, tag="sp")
                    nc.vector.tensor_add(sp[:], pa[:], pb[:])
                    first, last = k == 0, k == mt // 2 - 1
                    for s in range(spc):
                        ssl = slice(s * fd, (s + 1) * fd)
                        nc.tensor.matmul(
                            p_den[:, ssl], ones_sb, sp[:, ssl],
                            start=first, stop=last,
                        )

                scores(0)
                for m in range(1, mt):
                    scores(m)
                    accum(m - 1)
                    if m >= 2 and m % 2 == 0:
                        pair((m - 2) // 2)
                    elif m >= 3 and events:
                        events.pop(0)()
                accum(mt - 1)
                pair(mt // 2 - 1)

                rec = rpool.tile([C, ch], F32, tag="rec")
                nc.vector.reciprocal_approx_fast(rec[:], p_den[:])
                osl = slice(cidx * ch, (cidx + 1) * ch)
                nc.vector.tensor_mul(o_sb[:, osl], p_out[:], rec[:])

                def bias_add(c=cidx):
                    bsl = slice(c * ch, (c + 1) * ch)
                    nc.vector.tensor_scalar_add(o_sb[:, bsl], o_sb[:, bsl], bv_sb)

                events.append(bias_add)
                if post_chunk is not None:
                    events.extend(post_chunk(cidx))

        attention(o1_sb, Q1_sb, K1_sb, VT1_sb, b_sb["bv1"])

        def fuse_and_project(cidx):
            # fused' = (o1+o2) + t*(o1-o2);  out = (0.5*wp)^T.T @ fused' + bp
            # Emitted as small thunks (512 cols each) via the event queue so
            # the serial DVE chain overlaps the next chunk's attention loop.
            thunks = []
            for s in range(spc):
                sl = slice(cidx * ch + s * fd, cidx * ch + (s + 1) * fd)

                def _sub(sl=sl):
                    nc.vector.tensor_sub(d_sb[:, sl], o1_sb[:, sl], o2_sb[:, sl])

                def _sum(sl=sl):
                    nc.vector.tensor_add(o1_sb[:, sl], o1_sb[:, sl], o2_sb[:, sl])

                def _gate(sl=sl):
                    nc.vector.tensor_mul(d_sb[:, sl], d_sb[:, sl], t_sb[:, sl])

                def _fuse(sl=sl):
                    nc.vector.tensor_add(o1_sb[:, sl], o1_sb[:, sl], d_sb[:, sl])

                def _proj(sl=sl):
                    ps = ps_pool.tile([C, ch], F32, tag="ps")
                    nc.tensor.matmul(
                        ps[:, :fd], w_sb["wpTs"], o1_sb[:, sl], start=True, stop=True
                    )
                    st = stpool.tile([C, fd], F32, tag="st")
                    nc.vector.tensor_scalar_add(st[:], ps[:, :fd], b_sb["bp"])
                    nc.sync.dma_start(out=out_d[:, sl], in_=st[:])

                thunks += [_sub, _sum, _gate, _fuse, _proj]
            return thunks

        attention(o2_sb, Q2_sb, K2_sb, VT2_sb, b_sb["bv2"], post_chunk=fuse_and_project)
        while events:
            events.pop(0)()

    nc.compile()
    return nc


def make_in_maps(hsi, msi, weights, n_cores=8):
    """Host-side sharding: core i handles (b=i//2, half=i%2); the token axis is
    rotated so the core's queries are columns [0, NQ)."""
    B = hsi.shape[0]
    hsi = np.ascontiguousarray(hsi.reshape(B, C, N_TOK), dtype=np.float32)
    msi = np.ascontiguousarray(msi.reshape(B, C, N_TOK), dtype=np.float32)
    in_maps = []
    for core in range(n_cores):
        b, h = core // 2, core % 2
        if h == 0:
            x_h, x_m = hsi[b], msi[b]
        else:
            x_h = np.concatenate([hsi[b][:, NQ:], hsi[b][:, :NQ]], axis=1)
            x_m = np.concatenate([msi[b][:, NQ:], msi[b][:, :NQ]], axis=1)
        m = {"x_h": np.ascontiguousarray(x_h), "x_m": np.ascontiguousarray(x_m)}
        m.update(weights)
        in_maps.append(m)
    return in_maps


def make_weight_map(
    wq1, bq1, wk1, bk1, wv1, bv1, wq2, bq2, wk2, bk2, wv2, bv2, wg, bg, wp, bp
):
    f = np.float32
    col = lambda v: np.ascontiguousarray(np.asarray(v, f).reshape(C, 1))
    tr = lambda w: np.ascontiguousarray(np.asarray(w, f).T)
    w = {
        "wq1T": tr(wq1), "wk1T": tr(wk1), "wv1T": tr(wv1),
        "wq2T": tr(wq2), "wk2T": tr(wk2), "wv2T": tr(wv2),
        "wgaT": tr(np.asarray(wg, f)[:, :C]),
        "wgbT": tr(np.asarray(wg, f)[:, C:]),
        "wpTs": tr(0.5 * np.asarray(wp, f)),
    }
    b = {
        "bq1": col(bq1), "bk1": col(bk1), "bq2": col(bq2), "bk2": col(bk2),
        "bv1": col(bv1), "bv2": col(bv2), "bgh": col(0.5 * np.asarray(bg, f)),
        "bp": col(bp),
    }
    wpack = np.concatenate(
        [w[n] for n in WEIGHT_NAMES] + [np.ones((C, C), f)], axis=1
    )
    bpack = np.concatenate([b[n] for n in BIAS_NAMES], axis=1)
    return {
        "wpack": np.ascontiguousarray(wpack),
        "bpack": np.ascontiguousarray(bpack),
    }


_NC_CACHE = {}


def _get_program():
    if "nc" not in _NC_CACHE:
        _NC_CACHE["nc"] = build_program()
    return _NC_CACHE["nc"]


def run_on_cores(in_maps, trace=False, **kwargs):
    from concourse.bass_utils import run_bass_kernel_spmd

    nc = _get_program()
    return run_bass_kernel_spmd(
        nc, in_maps, core_ids=list(range(len(in_maps))), trace=trace, **kwargs
    )


def kernel(
    hsi, msi, wq1, bq1, wk1, bk1, wv1, bv1, wq2, bq2, wk2, bk2, wv2, bv2,
    wg, bg, wp, bp,
):
    B, _, H, W = hsi.shape
    weights = make_weight_map(
        wq1, bq1, wk1, bk1, wv1, bv1, wq2, bq2, wk2, bk2, wv2, bv2, wg, bg, wp, bp
    )
    in_maps = make_in_maps(np.asarray(hsi), np.asarray(msi), weights)
    res = run_on_cores(in_maps)
    out = np.zeros((B, C, N_TOK), dtype=np.float32)
    for core in range(8):
        b, h = core // 2, core % 2
        out[b][:, h * NQ : (h + 1) * NQ] = res.results[core]["out"]
    return out.reshape(B, C, H, W)


# revision 11
# speedup vs baseline: 1.4864x; 1.0080x over previous
"""CrossAttentionFusion Trainium2 kernel.

Problem (per batch element b of 4, C=128 channels, N=4096 tokens):
    Q1 = wq1@hsi+bq1; K1 = wk1@msi+bk1; V1 = wv1@msi+bv1   (1x1 convs)
    Q2 = wq2@msi+bq2; K2 = wk2@hsi+bk2; V2 = wv2@hsi+bv2
    out1 = attn(Q1,K1,V1); out2 = attn(Q2,K2,V2)           (softmax over keys)
    g = sigmoid(wg@[hsi;msi]+bg)
    out = wp@(g*out1 + (1-g)*out2) + bp

Sharding: 8 cores = (b, query-half). Each core computes 2048 query columns
for one batch element; keys/values span all 4096 tokens. Host permutes the
token axis per core so its queries are the first 2048 columns (key order is
irrelevant to attention sums), so the SPMD program is offset-free.

Core dataflow (transposed attention, keys on partitions):
    sT[m,n] = K[:,m]^T Q[:,n]        via matmul(lhsT=K tile, rhs=Q)
    pT = exp(scale*sT)               ACT, direct from PSUM
    den[n] = sum_m pT[m,n]           via matmul(lhsT=ones[128,128]) -> bcast rows
    outU[c,n] = sum_m VT[m,c] pT[m,n] accumulated over key tiles in PSUM
    out = outU * (1/den) + bv        (V-bias folds through softmax exactly)
Gate uses sigmoid(z) = 0.5*tanh(0.5 z)+0.5; the 0.5 factors fold into wp.
Matmuls run as float32r (full-rate fp32); the V-producing convs stay
float32 (exact) since their N=128 free dim gets no fp32r speedup anyway.
"""

import sys

if "/opt/trn_rl_repo" not in sys.path:
    sys.path.insert(0, "/opt/trn_rl_repo")

from contextlib import ExitStack

import numpy as np

import concourse.bacc as bacc
import concourse.bass as bass  # noqa: F401
import concourse.tile as tile
from concourse import mybir

F32 = mybir.dt.float32
F32R = mybir.dt.float32r
C = 128
N_TOK = 4096
NQ = 2048
FD = 512  # matmul moving-operand max for 4-byte dtypes
CH = 1024  # query-chunk width (PSUM accumulator width)
SCALE = 1.0 / float(np.sqrt(np.float32(C)))

WEIGHT_NAMES = ["wq1T", "wk1T", "wv1T", "wq2T", "wk2T", "wv2T", "wgaT", "wgbT", "wpTs"]
BIAS_NAMES = ["bq1", "bk1", "bq2", "bk2", "bv1", "bv2", "bgh", "bp"]


def _r(ap):
    return ap.bitcast(F32R)


def build_program(n_tok=N_TOK, nq=NQ, ch=CH, fd=FD):
    mt = n_tok // 128  # key tiles
    nch = nq // ch  # query chunks
    spc = ch // fd  # matmul slices per chunk
    vtg = ch // 128  # VT tiles per eviction group

    nc = bacc.Bacc("TRN2", target_bir_lowering=False, debug=False)
    din = {}
    for name in ["x_h", "x_m"]:
        din[name] = nc.dram_tensor(name, [C, n_tok], F32, kind="ExternalInput").ap()
    nw = len(WEIGHT_NAMES) + 1  # +1 for the all-ones block
    din["wpack"] = nc.dram_tensor("wpack", [C, nw * C], F32, kind="ExternalInput").ap()
    din["bpack"] = nc.dram_tensor(
        "bpack", [C, len(BIAS_NAMES)], F32, kind="ExternalInput"
    ).ap()
    out_d = nc.dram_tensor("out", [C, nq], F32, kind="ExternalOutput").ap()

    with ExitStack() as ctx:
        tc = ctx.enter_context(tile.TileContext(nc))
        const = ctx.enter_context(tc.tile_pool(name="const", bufs=1))
        big = ctx.enter_context(tc.tile_pool(name="big", bufs=1))
        ppool = ctx.enter_context(tc.tile_pool(name="ppool", bufs=4))
        rpool = ctx.enter_context(tc.tile_pool(name="rpool", bufs=1))
        sppool = ctx.enter_context(tc.tile_pool(name="sppool", bufs=2))
        stpool = ctx.enter_context(tc.tile_pool(name="stpool", bufs=2))
        ps_pool = ctx.enter_context(tc.tile_pool(name="ps", bufs=2, space="PSUM"))
        pacc_pool = ctx.enter_context(tc.tile_pool(name="pacc", bufs=2, space="PSUM"))

        # constants in: one packed DMA for weights (gpsimd SWDGE ring),
        # one for biases, so the head isn't serialized on per-tensor DMAs
        wpack_sb = const.tile([C, nw * C], F32R, name="wpack")
        nc.gpsimd.dma_start(out=wpack_sb[:], in_=_r(din["wpack"][:]))
        bpack_sb = const.tile([C, len(BIAS_NAMES)], F32, name="bpack")
        nc.gpsimd.dma_start(out=bpack_sb[:], in_=din["bpack"][:])
        w_sb = {
            name: wpack_sb[:, i * C : (i + 1) * C]
            for i, name in enumerate(WEIGHT_NAMES)
        }
        ones_sb = wpack_sb[:, len(WEIGHT_NAMES) * C :]
        b_sb = {name: bpack_sb[:, i : i + 1] for i, name in enumerate(BIAS_NAMES)}

        # activations in, chunked so convs start early; the two inputs go to
        # the two independent HWDGE rings (SP + ACT) to halve the head time
        xh_sb = big.tile([C, n_tok], F32R, name="xh")
        xm_sb = big.tile([C, n_tok], F32R, name="xm")
        for j in range(n_tok // fd):
            sl = slice(j * fd, (j + 1) * fd)
            nc.scalar.dma_start(out=xh_sb[:, sl], in_=_r(din["x_h"][:, sl]))
            nc.sync.dma_start(out=xm_sb[:, sl], in_=_r(din["x_m"][:, sl]))

        K1_sb = big.tile([C, n_tok], F32R, name="K1")
        K2_sb = big.tile([C, n_tok], F32R, name="K2")
        VT1_sb = big.tile([C, n_tok], F32R, name="VT1")
        VT2_sb = big.tile([C, n_tok], F32R, name="VT2")
        Q1_sb = big.tile([C, nq], F32R, name="Q1")
        Q2_sb = big.tile([C, nq], F32R, name="Q2")
        o1_sb = big.tile([C, nq], F32R, name="o1")
        o2_sb = big.tile([C, nq], F32R, name="o2")
        t_sb = big.tile([C, nq], F32R, name="t")
        tb_sb = big.tile([C, nq], F32R, name="tb")
        d_sb = big.tile([C, nq], F32R, name="d")

        def conv(dst_sb, wT_sb, x_sb, cols, bias_sb):
            # dst[:, :cols] = wT.T @ x[:, :cols] (+ bias per channel).
            # Evictions alternate between DVE and ACT so the PSUM drain
            # keeps up with the matmul stream during the conv phase.
            for j in range(cols // ch):
                ps = ps_pool.tile([C, ch], F32, tag="ps")
                for s in range(spc):
                    sl = slice(j * ch + s * fd, j * ch + (s + 1) * fd)
                    nc.tensor.matmul(
                        ps[:, s * fd : (s + 1) * fd],
                        wT_sb,
                        x_sb[:, sl],
                        start=True,
                        stop=True,
                    )
                dsl = slice(j * ch, (j + 1) * ch)
                if j % 2 == 0:
                    nc.vector.tensor_scalar_add(dst_sb[:, dsl], ps[:], bias_sb)
                else:
                    nc.scalar.activation(
                        dst_sb[:, dsl],
                        ps[:],
                        mybir.ActivationFunctionType.Identity,
                        bias=bias_sb,
                    )

        def vt_conv(dst_sb, x_sb, wvT_sb):
            # dst tile j holds V^T rows for tokens [128j, 128j+128): [tok, chan]
            for g in range(mt // vtg):
                ps = ps_pool.tile([C, ch], F32, tag="ps")
                for u in range(vtg):
                    j = g * vtg + u
                    nc.tensor.matmul(
                        ps[:, u * 128 : (u + 1) * 128],
                        x_sb[:, j * 128 : (j + 1) * 128],
                        wvT_sb,
                        start=True,
                        stop=True,
                    )
                nc.scalar.copy(dst_sb[:, g * ch : (g + 1) * ch], ps[:])

        conv(K1_sb, w_sb["wk1T"], xm_sb, n_tok, b_sb["bk1"])
        conv(Q1_sb, w_sb["wq1T"], xh_sb, nq, b_sb["bq1"])
        conv(K2_sb, w_sb["wk2T"], xh_sb, n_tok, b_sb["bk2"])
        conv(Q2_sb, w_sb["wq2T"], xm_sb, nq, b_sb["bq2"])

        # gate: t = tanh(0.5*(wgA@xq_h + wgB@xq_m) + 0.5*bg)
        for j in range(nq // ch):
            ps = ps_pool.tile([C, ch], F32, tag="ps")
            for s in range(spc):
                sl = slice(j * ch + s * fd, j * ch + (s + 1) * fd)
                psl = ps[:, s * fd : (s + 1) * fd]
                nc.tensor.matmul(
                    psl, w_sb["wgaT"], xh_sb[:, sl], start=True, stop=False
                )
                nc.tensor.matmul(
                    psl, w_sb["wgbT"], xm_sb[:, sl], start=False, stop=True
                )
            nc.scalar.activation(
                t_sb[:, j * ch : (j + 1) * ch],
                ps[:],
                mybir.ActivationFunctionType.Tanh,
                bias=b_sb["bgh"],
                scale=0.5,
            )
        # gate weights for the 3-op fusion: t <- 1+tanh(...), tb <- 1-tanh(...)
        # (fused = 0.5*[o1*(1+t') + o2*(1-t')] with the 0.5 folded into wp)
        for j in range(nq // ch):
            sl = slice(j * ch, (j + 1) * ch)
            nc.vector.tensor_scalar(
                tb_sb[:, sl], t_sb[:, sl], -1.0, 1.0,
                mybir.AluOpType.mult, mybir.AluOpType.add,
            )
            nc.vector.tensor_scalar_add(t_sb[:, sl], t_sb[:, sl], 1.0)
        vt_conv(VT1_sb, xm_sb, w_sb["wv1T"])
        vt_conv(VT2_sb, xh_sb, w_sb["wv2T"])

        # Deferred-work queue: thunks drained one per two m-iterations inside
        # the attention loops, so serial DVE chains (normalize bias, gated
        # fusion, projection) never stall the in-order PE queue.
        events = []

        def attention(o_sb, Q_sb, K_sb, VT_sb, bv_sb, post_chunk=None):
            # Software-pipelined: scores for key-tile m+1 are emitted before
            # the PV matmuls of tile m, so the PE never sits waiting on
            # ACT's exp of tile m (exp overlaps the next matmuls).
            for cidx in range(nch):
                p_out = pacc_pool.tile([C, ch], F32, tag="acc")
                p_den = pacc_pool.tile([C, ch], F32, tag="acc")
                pts = {}

                def scores(m):
                    ksl = slice(m * 128, (m + 1) * 128)
                    ps = ps_pool.tile([C, ch], F32, tag="ps")
                    for s in range(spc):
                        qsl = slice(cidx * ch + s * fd, cidx * ch + (s + 1) * fd)
                        nc.tensor.matmul(
                            ps[:, s * fd : (s + 1) * fd],
                            K_sb[:, ksl],
                            Q_sb[:, qsl],
                            start=True,
                            stop=True,
                        )
                    pt = ppool.tile([C, ch], F32R, tag="pt")
                    nc.scalar.activation(
                        pt[:], ps[:], mybir.ActivationFunctionType.Exp, scale=SCALE
                    )
                    pts[m] = pt

                def accum(m):
                    # PV accumulation only; the softmax denominator is fed by
                    # pair() below (DVE pair-sums halve the ones-matmul count)
                    ksl = slice(m * 128, (m + 1) * 128)
                    pt = pts[m]
                    first, last = m == 0, m == mt - 1
                    for s in range(spc):
                        ssl = slice(s * fd, (s + 1) * fd)
                        nc.tensor.matmul(
                            p_out[:, ssl], VT_sb[:, ksl], pt[:, ssl],
                            start=first, stop=last,
                        )

                def pair(k):
                    pa, pb = pts.pop(2 * k), pts.pop(2 * k + 1)
                    sp = sppool.tile([C, ch], F32R, tag="sp")
                    nc.vector.tensor_add(sp[:], pa[:], pb[:])
                    first, last = k == 0, k == mt // 2 - 1
                    for s in range(spc):
                        ssl = slice(s * fd, (s + 1) * fd)
                        nc.tensor.matmul(
                            p_den[:, ssl], ones_sb, sp[:, ssl],
                            start=first, stop=last,
                        )

                scores(0)
                for m in range(1, mt):
                    scores(m)
                    accum(m - 1)
                    if m >= 2 and m % 2 == 0:
                        pair((m - 2) // 2)
                    elif m >= 3 and events:
                        events.pop(0)()
                accum(mt - 1)
                pair(mt // 2 - 1)

                rec = rpool.tile([C, ch], F32, tag="rec")
                nc.vector.reciprocal_approx_fast(rec[:], p_den[:])
                osl = slice(cidx * ch, (cidx + 1) * ch)
                nc.vector.tensor_mul(o_sb[:, osl], p_out[:], rec[:])

                def bias_add(c=cidx):
                    bsl = slice(c * ch, (c + 1) * ch)
                    nc.vector.tensor_scalar_add(o_sb[:, bsl], o_sb[:, bsl], bv_sb)

                events.append(bias_add)
                if post_chunk is not None:
                    events.extend(post_chunk(cidx))

        attention(o1_sb, Q1_sb, K1_sb, VT1_sb, b_sb["bv1"])

        def fuse_and_project(cidx):
            # fused' = (o1+o2) + t*(o1-o2);  out = (0.5*wp)^T.T @ fused' + bp
            # Emitted as small thunks (512 cols each) via the event queue so
            # the serial DVE chain overlaps the next chunk's attention loop.
            thunks = []
            for s in range(spc):
                sl = slice(cidx * ch + s * fd, cidx * ch + (s + 1) * fd)

                def _gb(sl=sl):
                    nc.vector.tensor_mul(d_sb[:, sl], o2_sb[:, sl], tb_sb[:, sl])

                def _ga(sl=sl):
                    nc.vector.tensor_mul(o1_sb[:, sl], o1_sb[:, sl], t_sb[:, sl])

                def _fuse(sl=sl):
                    nc.vector.tensor_add(o1_sb[:, sl], o1_sb[:, sl], d_sb[:, sl])

                def _proj(sl=sl):
                    ps = ps_pool.tile([C, ch], F32, tag="ps")
                    nc.tensor.matmul(
                        ps[:, :fd], w_sb["wpTs"], o1_sb[:, sl], start=True, stop=True
                    )
                    st = stpool.tile([C, fd], F32, tag="st")
                    nc.vector.tensor_scalar_add(st[:], ps[:, :fd], b_sb["bp"])
                    nc.sync.dma_start(out=out_d[:, sl], in_=st[:])

                thunks += [_gb, _ga, _fuse, _proj]
            return thunks

        attention(o2_sb, Q2_sb, K2_sb, VT2_sb, b_sb["bv2"], post_chunk=fuse_and_project)
        while events:
            events.pop(0)()

    nc.compile()
    return nc


def make_in_maps(hsi, msi, weights, n_cores=8):
    """Host-side sharding: core i handles (b=i//2, half=i%2); the token axis is
    rotated so the core's queries are columns [0, NQ)."""
    B = hsi.shape[0]
    hsi = np.ascontiguousarray(hsi.reshape(B, C, N_TOK), dtype=np.float32)
    msi = np.ascontiguousarray(msi.reshape(B, C, N_TOK), dtype=np.float32)
    in_maps = []
    for core in range(n_cores):
        b, h = core // 2, core % 2
        if h == 0:
            x_h, x_m = hsi[b], msi[b]
        else:
            x_h = np.concatenate([hsi[b][:, NQ:], hsi[b][:, :NQ]], axis=1)
            x_m = np.concatenate([msi[b][:, NQ:], msi[b][:, :NQ]], axis=1)
        m = {"x_h": np.ascontiguousarray(x_h), "x_m": np.ascontiguousarray(x_m)}
        m.update(weights)
        in_maps.append(m)
    return in_maps


def make_weight_map(
    wq1, bq1, wk1, bk1, wv1, bv1, wq2, bq2, wk2, bk2, wv2, bv2, wg, bg, wp, bp
):
    f = np.float32
    col = lambda v: np.ascontiguousarray(np.asarray(v, f).reshape(C, 1))
    tr = lambda w: np.ascontiguousarray(np.asarray(w, f).T)
    w = {
        "wq1T": tr(wq1), "wk1T": tr(wk1), "wv1T": tr(wv1),
        "wq2T": tr(wq2), "wk2T": tr(wk2), "wv2T": tr(wv2),
        "wgaT": tr(np.asarray(wg, f)[:, :C]),
        "wgbT": tr(np.asarray(wg, f)[:, C:]),
        "wpTs": tr(0.5 * np.asarray(wp, f)),
    }
    b = {
        "bq1": col(bq1), "bk1": col(bk1), "bq2": col(bq2), "bk2": col(bk2),
        "bv1": col(bv1), "bv2": col(bv2), "bgh": col(0.5 * np.asarray(bg, f)),
        "bp": col(bp),
    }
    wpack = np.concatenate(
        [w[n] for n in WEIGHT_NAMES] + [np.ones((C, C), f)], axis=1
    )
    bpack = np.concatenate([b[n] for n in BIAS_NAMES], axis=1)
    return {
        "wpack": np.ascontiguousarray(wpack),
        "bpack": np.ascontiguousarray(bpack),
    }


_NC_CACHE = {}


def _get_program():
    if "nc" not in _NC_CACHE:
        _NC_CACHE["nc"] = build_program()
    return _NC_CACHE["nc"]


def run_on_cores(in_maps, trace=False, **kwargs):
    from concourse.bass_utils import run_bass_kernel_spmd

    nc = _get_program()
    return run_bass_kernel_spmd(
        nc, in_maps, core_ids=list(range(len(in_maps))), trace=trace, **kwargs
    )


def kernel(
    hsi, msi, wq1, bq1, wk1, bk1, wv1, bv1, wq2, bq2, wk2, bk2, wv2, bv2,
    wg, bg, wp, bp,
):
    B, _, H, W = hsi.shape
    weights = make_weight_map(
        wq1, bq1, wk1, bk1, wv1, bv1, wq2, bq2, wk2, bk2, wv2, bv2, wg, bg, wp, bp
    )
    in_maps = make_in_maps(np.asarray(hsi), np.asarray(msi), weights)
    res = run_on_cores(in_maps)
    out = np.zeros((B, C, N_TOK), dtype=np.float32)
    for core in range(8):
        b, h = core // 2, core % 2
        out[b][:, h * NQ : (h + 1) * NQ] = res.results[core]["out"]
    return out.reshape(B, C, H, W)
